# revision 23
# baseline (speedup 1.0000x reference)
"""AttentionPruneViT-Small Trainium2 kernel (Bass/Tile), data-parallel over
batch on 8 NeuronCores (8 images per core).

Self-contained: hardcodes all shapes; host side patchifies the input, folds
LN affines into adjacent weights, lays out weights for the device, runs the
Bass kernel on cores 0-7 and reassembles the [64, 100] output.

Numerics: fp32r matmuls (11-bit mantissa) everywhere except the attention
AV product (exp weights and V in fp16) and fc2 (gelu activations / weights
in fp16) -- validated on this input distribution. Softmax without
max-subtraction (scores are tiny). Token pruning is done by masking pruned
keys out of attention (exp bias of -1e30); after the last prune (layer 6)
the 120 surviving tokens are physically compacted into a single 128-token
chunk via a permutation matmul, so layers 7-11 run single-chunk.

Structure: layers are emitted per image-pair (LN stats/normalize for pair
p+1 overlap attention/MLP matmuls of pair p), keeping the PE dense so the
HAM clock gate stays at full rate.
"""
import os
import numpy as np
from contextlib import ExitStack

import concourse.bass as bass
import concourse.mybir as mybir
import concourse.tile as tile
from concourse import bacc
from concourse.bass_utils import run_bass_kernel_spmd

F32 = mybir.dt.float32
F32R = mybir.dt.float32r
BF16 = mybir.dt.bfloat16
F16 = mybir.dt.float16
AF = mybir.ActivationFunctionType
OP = mybir.AluOpType

# model constants
B = 64
C = 384
HEADS = 6
HD = 64
MLP = 1536
LAYERS = 12
NPATCH = 196
NTOK = 197
NCLS = 100
EPS = 1e-6
PRUNE = {2: 20, 4: 27, 6: 30}   # layer -> number of tokens dropped
SCALE = HD ** -0.5
NKEEP = 120                     # tokens surviving all prunes
COMPACT_AT = 6

# per-core geometry
IMGS = 8                 # images per core
PAIRS = IMGS // 2
NEG = -1e30

# geometry A: layers 0..6 (197 tokens, two chunks per image)
W_A = 208
CH_A = [(0, 128), (128, 80)]
QWIN_A = [0, 160]        # S-matmul rhs window start per image-in-pair
QSH_A = [0, 48]          # query column of token 0 within the window
# geometry B: layers 7..11 (120 tokens, one chunk per image)
W_B = 128
CH_B = [(0, 128)]
QWIN_B = [0, 0]
QSH_B = [0, 128]

N_LAYERS_BUILD = int(os.environ.get("VIT_LAYERS", str(LAYERS)))
NO_COMPACT = os.environ.get("VIT_NOCOMPACT", "") == "1"


def _rsqrt(nc, pool, out, var, eps):
    """out = 1/sqrt(var + eps): DVE add+reciprocal, then Sqrt on ACT."""
    P, n = var.shape[0], var.shape[1]
    iv = pool.tile([P, n], F32, tag="rsq_iv")
    nc.vector.tensor_scalar_add(iv[:], var[:], float(eps))
    nc.vector.reciprocal(iv[:], iv[:])
    nc.scalar.activation(out[:], iv[:], AF.Sqrt)


def build_kernel(n_layers=N_LAYERS_BUILD):
    nc = bacc.Bacc(target_bir_lowering=False)
    P = 128

    # ---------------- DRAM I/O ----------------
    xp = nc.dram_tensor("xp", [6, 128, IMGS, W_A], F32R, kind="ExternalInput")
    pw = nc.dram_tensor("pw", [6, 128, C], F32R, kind="ExternalInput")
    h0b = nc.dram_tensor("h0b", [2, 128, C], F32, kind="ExternalInput")
    mb0 = nc.dram_tensor("mb0", [2, 128], F32, kind="ExternalInput")
    wqk = nc.dram_tensor("wqk", [LAYERS, 3, 128, 768], F32R, kind="ExternalInput")
    bqk = nc.dram_tensor("bqk", [LAYERS, 6, 128], F32, kind="ExternalInput")
    wv = nc.dram_tensor("wv", [LAYERS, 3, 128, 396], F32R, kind="ExternalInput")
    bv = nc.dram_tensor("bv", [LAYERS, 1, 396], F32, kind="ExternalInput")
    wp = nc.dram_tensor("wp", [LAYERS, 3, 128, C], F32R, kind="ExternalInput")
    bp = nc.dram_tensor("bp", [LAYERS, 1, C], F32R, kind="ExternalInput")
    w1 = nc.dram_tensor("w1", [LAYERS, 3, 128, MLP], F32R, kind="ExternalInput")
    b1 = nc.dram_tensor("b1", [LAYERS, 1, MLP], F32, kind="ExternalInput")
    w2 = nc.dram_tensor("w2", [LAYERS, 12, 128, C], F16, kind="ExternalInput")
    b2 = nc.dram_tensor("b2", [LAYERS, 1, C], F16, kind="ExternalInput")
    wh = nc.dram_tensor("wh", [3, 128, NCLS], F32R, kind="ExternalInput")
    bh = nc.dram_tensor("bh", [1, NCLS], F32R, kind="ExternalInput")
    out = nc.dram_tensor("out", [IMGS, NCLS], F32, kind="ExternalOutput")
    # DRAM scratch for cross-partition bounces (prune bookkeeping)
    sc_dram = nc.dram_tensor("sc_dram", [IMGS, 2 * 128], F32)
    dm_dram = nc.dram_tensor("dm_dram", [IMGS, 2 * 128], F32)
    cls_dram = nc.dram_tensor("cls_dram", [IMGS, C], F32)

    with tile.TileContext(nc) as tc, ExitStack() as ctx:
        cpool = ctx.enter_context(tc.tile_pool(name="const", bufs=1))
        spool = ctx.enter_context(tc.tile_pool(name="stats", bufs=3))
        wpool = ctx.enter_context(tc.tile_pool(name="weights", bufs=2))
        wpool1 = ctx.enter_context(tc.tile_pool(name="weights1", bufs=1))
        apool = ctx.enter_context(tc.tile_pool(name="acts", bufs=3))
        qkpool = ctx.enter_context(tc.tile_pool(name="qkp", bufs=1))
        prpool = ctx.enter_context(tc.tile_pool(name="prp", bufs=1))
        vpool = ctx.enter_context(tc.tile_pool(name="vtile", bufs=5))
        ppool = ctx.enter_context(tc.tile_pool(name="ptile", bufs=8))
        opool = ctx.enter_context(tc.tile_pool(name="otile", bufs=3))
        xpool = ctx.enter_context(tc.tile_pool(name="xfm", bufs=3))
        gpool = ctx.enter_context(tc.tile_pool(name="gtile", bufs=1))
        ps1 = ctx.enter_context(tc.tile_pool(name="ps1", bufs=6, space="PSUM"))
        pstr = ctx.enter_context(tc.tile_pool(name="pstr", bufs=2, space="PSUM"))

        # persistent state
        h = cpool.tile([P, IMGS, 2, C], F32)              # residual stream
        mb = cpool.tile([P, IMGS, 2], F32)                # attention key bias
        ident = cpool.tile([P, P], F32R)
        ones_r = cpool.tile([1, P], F32R)
        ones_b = cpool.tile([1, P], F16)

        from concourse.masks import make_identity, make_upper_triangular
        identF = prpool.tile([P, P], F32, tag="identF")
        make_identity(nc, identF[:])
        nc.vector.tensor_copy(ident[:], identF[:])
        nc.vector.memset(h[:], 0.0)
        nc.vector.memset(ones_r[:].bitcast(F32), 1.0)
        nc.vector.memset(ones_b[:], 1.0)
        for b in range(IMGS):
            nc.sync.dma_start(mb[:, b, :], mb0.ap().rearrange("c p -> p c"))

        # compaction constants
        trU = cpool.tile([P, P], F32)       # trU[s, t] = 1 if s <= t
        make_upper_triangular(nc, trU[:], 1.0, diag=True)
        ones128 = cpool.tile([P, P], F32)
        nc.vector.memset(ones128[:], 1.0)
        io_row = cpool.tile([1, P], F32)
        nc.gpsimd.iota(io_row[:], [[1, P]], base=1, channel_multiplier=0,
                       allow_small_or_imprecise_dtypes=True)
        io1b = cpool.tile([P, P], F32)
        nc.gpsimd.partition_broadcast(io1b[:], io_row[:])

        h0b_t = prpool.tile([P, 2, C], F32, tag="h0b")
        nc.sync.dma_start(h0b_t[:], h0b.ap().rearrange("c p f -> p c f"))

        # ---------------- patch embed ----------------
        pw_t = cpool.tile([P, 6, C], F32R)
        nc.sync.dma_start(pw_t[:], pw.ap().rearrange("k p f -> p k f"))
        for b in range(IMGS):
            xp_t = gpool.tile([P, 6, W_A], F32R, tag="g")
            nc.sync.dma_start(xp_t[:], xp.ap()[:, :, b, :].rearrange("k p t -> p k t"))
            for c, (off, wd) in enumerate(CH_A):
                ps = ps1.tile([P, 512], F32, tag="ps1")
                acc = ps[:wd, :C]
                for kt in range(6):
                    nc.tensor.matmul(acc, xp_t[:, kt, off:off + wd],
                                     pw_t[:, kt, :], start=(kt == 0),
                                     stop=(kt == 5))
                nc.vector.tensor_tensor(h[:wd, b, c, :], acc,
                                        h0b_t[:wd, c, :], OP.add)

        # ---------------- per-layer helpers ----------------
        def ln_stats(p, CHg):
            """LN statistics for pair p (DVE only): returns (rstd, nmean)."""
            mv = spool.tile([P, 2, 2, 2], F32, tag="mv")
            nc.vector.memset(mv[:], 1.0)
            for bj in range(2):
                b = 2 * p + bj
                for c, (off, wd) in enumerate(CHg):
                    s6 = spool.tile([P, 6], F32, tag="s6")
                    nc.vector.bn_stats(s6[:wd, :], h[:wd, b, c, :])
                    nc.vector.bn_aggr(mv[:wd, bj, c, :], s6[:wd, :])
            rstd = spool.tile([P, 4], F32, tag="rstd")
            _rsqrt(nc, spool, rstd,
                   mv[:].rearrange("p b c s -> p (b c) s")[:, :, 1], EPS)
            nmean = spool.tile([P, 4], F32, tag="nmean")
            nc.vector.scalar_tensor_tensor(
                nmean[:], mv[:].rearrange("p b c s -> p (b c) s")[:, :, 0],
                -1.0, rstd[:], OP.mult, OP.mult)
            return rstd, nmean

        def ln_apply(p, CHg, Wg, xfm, rstd, nmean):
            """Normalize (ACT) + transpose (PE) into feature-major xfm."""
            for bj in range(2):
                b = 2 * p + bj
                for c, (off, wd) in enumerate(CHg):
                    xl = apool.tile([P, C], F32R, tag="xln")
                    i = bj * 2 + c
                    if i % 2 == 0:
                        nc.vector.tensor_scalar(
                            xl[:wd, :], h[:wd, b, c, :], rstd[:wd, i:i + 1],
                            nmean[:wd, i:i + 1], OP.mult, OP.add)
                    else:
                        nc.scalar.activation(
                            xl[:wd, :], h[:wd, b, c, :], AF.Identity,
                            bias=nmean[:wd, i:i + 1], scale=rstd[:wd, i:i + 1])
                    pt = pstr.tile([P, 512], F32, tag="pstr")
                    ptr = pt.bitcast(F32R)
                    for f in range(3):
                        nc.tensor.transpose(
                            ptr[:, f * P:f * P + wd], xl[:wd, f * P:(f + 1) * P],
                            ident[:wd, :wd])
                    srcv = ptr[:, :384].rearrange("p (f w) -> p f w", w=P)[:, :, :wd]
                    dst = xfm[:, :, bj * Wg + off:bj * Wg + off + wd]
                    if (bj * 2 + c) % 2 == 0:
                        nc.vector.tensor_copy(dst, srcv)
                    else:
                        nc.scalar.copy(dst, srcv)

        def attn_h1(li, p, CHg, Wg, QWINg, QSHg, xfm, wqk_t, bqk_t, wv_t,
                    bv_m):
            b0 = 2 * p
            PWg = 2 * Wg
            nch = len(CHg)
            # Q,K for the pair: per head one [128, PW] matmul (Q stacked
            # on partitions 0-63, K on 64-127), bias peeled off on ACT
            qt = qkpool.tile([64, 6, PW_MAX], F32R, tag="qt")
            kt_ = qkpool.tile([64, 6, PW_MAX], F32R, tag="kt")
            for m in range(6):
                pt = ps1.tile([P, 512], F32, tag="ps1")
                acc = pt[:, :PWg]
                for kt in range(3):
                    nc.tensor.matmul(acc, wqk_t[:, kt, m * 128:(m + 1) * 128],
                                     xfm[:, kt, :PWg],
                                     start=(kt == 0), stop=(kt == 2))
                nc.scalar.activation(qt[:, m, :PWg], acc[0:64, :], AF.Identity,
                                     bias=bqk_t[0:64, m:m + 1])
                nc.vector.tensor_scalar_add(kt_[:, m, :PWg], acc[64:128, :],
                                            bqk_t[64:128, m:m + 1])
            # V per image: token-major [tok, chunk, 6*66] bf16
            vts = []
            for j in (0, 1):
                vt = vpool.tile([P, 2, 396], F16, tag="vt")
                for c, (off, wd) in enumerate(CHg):
                    pv = ps1.tile([P, 512], F32, tag="ps1")
                    acc = pv[:wd, :396]
                    for kt in range(3):
                        nc.tensor.matmul(
                            acc, xfm[:, kt, j * Wg + off:j * Wg + off + wd],
                            wv_t[:, kt, :], start=(kt == 0), stop=(kt == 2))
                    nc.vector.tensor_tensor(vt[:wd, c, :], acc, bv_m[:wd, :],
                                            OP.add)
                vts.append(vt)
            # scores S^T + exp, image-interleaved; sps tiles are one PSUM
            # bank (2 heads) so S(next) overlaps exp(prev)
            ptss = [[None] * nch for _ in range(2)]
            e0s = [[None] * nch for _ in range(2)]
            for j in (0, 1):
                bi = b0 + j
                for c in range(nch):
                    ptss[j][c] = ppool.tile([P, 6, 256], F16, name="pt", tag="pt")
                    if li in PRUNE:
                        e0s[j][c] = prpool.tile([P, 6], F32, name=f"e0_{p % 2}{j}{c}", tag=f"e0_{p % 2}{j}{c}")
                for hp in range(3):
                    for c, (off, wd) in enumerate(CHg):
                        sps = ps1.tile([P, 2, 256], F32, name="sps", tag="ps1")
                        for hx in range(2):
                            hh = 2 * hp + hx
                            nc.tensor.matmul(
                                sps[:wd, hx, :],
                                kt_[:, hh, j * Wg + off:j * Wg + off + wd],
                                qt[:, hh, QWINg[j]:QWINg[j] + 256],
                                start=True, stop=True)
                        nc.scalar.activation(
                            ptss[j][c][:wd, 2 * hp:2 * hp + 2,
                                       QSHg[j]:QSHg[j] + Wg],
                            sps[:wd, :, QSHg[j]:QSHg[j] + Wg], AF.Exp,
                            bias=mb[:wd, bi, c:c + 1], scale=float(SCALE))
                        if li in PRUNE:
                            nc.scalar.activation(
                                e0s[j][c][:wd, 2 * hp:2 * hp + 2],
                                sps[:wd, :, QSHg[j]], AF.Exp,
                                bias=mb[:wd, bi, c:c + 1], scale=float(SCALE))
            return vts, ptss, e0s

        def attn_h2(li, p, CHg, Wg, QWINg, QSHg, state, wp_t, bp_t):
            b0 = 2 * p
            nch = len(CHg)
            vts, ptss, e0s = state
            # AV + denominators + proj, per image / query chunk
            for j in (0, 1):
                bi = b0 + j
                vt = vts[j]
                for qc, (qo, qw) in enumerate(CHg):
                    tps = ps1.tile([P, 512], F32, tag="ps1")
                    tview = tps[:, :396].rearrange("p (h c) -> p h c", c=66)
                    for hh in range(6):
                        for c, (off, wd) in enumerate(CHg):
                            nc.tensor.matmul(
                                tview[:qw, hh, :],
                                ptss[j][c][:wd, hh,
                                           QSHg[j] + qo:QSHg[j] + qo + qw],
                                vt[:wd, c, hh * 66:hh * 66 + 66],
                                start=(c == 0), stop=(c == nch - 1))
                    r = spool.tile([P, 6], F32, tag="rr")
                    nc.vector.reciprocal(r[:qw, :], tview[:qw, :, 64])
                    ot = opool.tile([P, 6, 64], F32R, tag="ot")
                    rap = r[:qw, :]
                    rb = bass.AP(rap.tensor, rap.offset,
                                 [list(x) for x in rap.ap] + [[0, 64]])
                    nc.vector.tensor_tensor(ot[:qw, :, :], tview[:qw, :, :64],
                                            rb, OP.mult)
                    # prune scores: weighted CLS column of exp tiles
                    if li in PRUNE and qc == 0:
                        wb = prpool.tile([P, 6], F32, tag="wb")
                        nc.gpsimd.partition_broadcast(wb[:], r[0:1, :])
                        sc = prpool.tile([P, 2], F32, tag="sc")
                        for c, (off, wd) in enumerate(CHg):
                            t6 = prpool.tile([P, 6], F32, tag="t6")
                            nc.vector.tensor_tensor(
                                t6[:wd, :], e0s[j][c][:wd, :], wb[:wd, :],
                                OP.mult)
                            nc.vector.reduce_sum(
                                sc[:wd, c:c + 1], t6[:wd, :],
                                axis=mybir.AxisListType.X)
                            nc.sync.dma_start(
                                sc_dram.ap()[bi, c * 128:c * 128 + wd],
                                sc[:wd, c])
                    # O^T -> feature-major via PE, then proj chunk
                    ofm = opool.tile([P, 3, P], F32R, tag="ofm")
                    pt2 = pstr.tile([P, 512], F32, tag="pstr")
                    pt2r = pt2.bitcast(F32R)
                    for f in range(3):
                        nc.tensor.transpose(
                            pt2r[:, f * P:f * P + qw],
                            ot[:qw, :, :].rearrange("p h d -> p (h d)")
                            [:, f * P:(f + 1) * P],
                            ident[:qw, :qw])
                    srco = pt2r[:, :384].rearrange("p (f w) -> p f w", w=P)[:, :, :qw]
                    if (j + qc) % 2 == 0:
                        nc.vector.tensor_copy(ofm[:, :, :qw], srco)
                    else:
                        nc.scalar.copy(ofm[:, :, :qw], srco)
                    pj = ps1.tile([P, 512], F32, tag="ps1")
                    acc = pj[:qw, :C]
                    for kt in range(3):
                        nc.tensor.matmul(acc, ofm[:, kt, :qw], wp_t[:, kt, :],
                                         start=(kt == 0), stop=(kt == 2))
                    yb = apool.tile([P, C], F32, tag="yb")
                    nc.vector.tensor_tensor(yb[:qw, :], acc, bp_t[:qw, :],
                                            OP.add)
                    nc.vector.tensor_tensor(h[:qw, bi, qc, :],
                                            h[:qw, bi, qc, :], yb[:qw, :],
                                            OP.add)

        def mlp_pair(p, CHg, Wg, xfm2, w1_t, b1f_t, w2_t, b2_t):
            b0 = 2 * p
            PWg = 2 * Wg
            g = gpool.tile([P, 12, PW_MAX], F16, tag="g")
            for m in range(12):
                f1 = ps1.tile([P, 512], F32, tag="ps1")
                acc = f1[:, :PWg]
                for kt in range(3):
                    nc.tensor.matmul(
                        acc, w1_t[:, kt, m * P:(m + 1) * P],
                        xfm2[:, kt, :PWg],
                        start=(kt == 0), stop=(kt == 2))
                nc.scalar.activation(g[:, m, :PWg], acc, AF.Gelu,
                                     bias=b1f_t[:, m:m + 1])
            for j in (0, 1):
                bi = b0 + j
                for c, (off, wd) in enumerate(CHg):
                    span = j * Wg + off
                    f2 = ps1.tile([P, 512], F32, tag="ps1")
                    acc = f2[:wd, :C]
                    for kt in range(12):
                        nc.tensor.matmul(acc, g[:, kt, span:span + wd],
                                         w2_t[:, kt, :],
                                         start=(kt == 0), stop=(kt == 11))
                    yb = apool.tile([P, C], F32, tag="yb")
                    nc.vector.tensor_tensor(yb[:wd, :], acc, b2_t[:wd, :],
                                            OP.add)
                    nc.vector.tensor_tensor(h[:wd, bi, c, :],
                                            h[:wd, bi, c, :], yb[:wd, :],
                                            OP.add)

        def mask_update(li):
            drop = PRUNE[li]
            scm = prpool.tile([IMGS, 256], F32, tag="scm")
            nc.sync.dma_start(scm[:], sc_dram.ap())
            # t = -1e9*(sc==0) - sc  over tokens 1..196
            tneg = prpool.tile([IMGS, 256], F32, tag="tneg")
            u = prpool.tile([IMGS, 256], F32, tag="uu")
            nc.vector.tensor_scalar(u[:, 1:NTOK], scm[:, 1:NTOK], 0.0, None,
                                    OP.is_equal)
            nc.vector.scalar_tensor_tensor(
                tneg[:, 1:NTOK], u[:, 1:NTOK], -1e9, scm[:, 1:NTOK],
                OP.mult, OP.subtract)
            m8 = prpool.tile([IMGS, 8], F32, tag="m8")
            left = drop
            while left > 0:
                k = min(8, left)
                nc.vector.max(m8[:], tneg[:, 1:NTOK])
                if k < 8:
                    nc.vector.memset(m8[:, k:], -2e30)
                nc.vector.match_replace(tneg[:, 1:NTOK], m8[:],
                                        tneg[:, 1:NTOK], NEG)
                left -= k
            dm = prpool.tile([IMGS, 256], F32, tag="dm")
            nc.vector.memset(dm[:], 0.0)
            nc.vector.tensor_scalar(dm[:, 1:NTOK], tneg[:, 1:NTOK], -1e29,
                                    None, OP.is_le)
            nc.sync.dma_start(dm_dram.ap(), dm[:])
            dmc = prpool.tile([P, IMGS, 2], F32, tag="dmc")
            for b in range(IMGS):
                nc.sync.dma_start(
                    dmc[:, b, :],
                    bass.AP(dm_dram, b * 256, [[1, 128], [128, 2]]))
            nc.vector.scalar_tensor_tensor(mb[:], dmc[:], NEG, mb[:],
                                           OP.mult, OP.add)

        def compact():
            """Gather the 120 kept tokens into chunk 0 (partitions 0..119)."""
            kq = prpool.tile([P, IMGS, 2], F32, tag="kq")
            nc.vector.tensor_scalar(kq[:], mb[:], 0.0, None, OP.is_equal)
            kqr = kq
            # ranks (inclusive prefix sums over token index) via PE
            rr = prpool.tile([P, IMGS, 2], F32, tag="rrk")
            pr0 = ps1.tile([P, 512], F32, tag="ps1")
            nc.tensor.matmul(pr0[:128, :IMGS], trU[:, :], kqr[:, :, 0],
                             start=True, stop=True)
            nc.vector.tensor_copy(rr[:, :, 0], pr0[:128, :IMGS])
            pr1 = ps1.tile([P, 512], F32, tag="ps1")
            nc.tensor.matmul(pr1[:80, :IMGS], ones128[:, :80], kqr[:, :, 0],
                             start=True, stop=False)
            nc.tensor.matmul(pr1[:80, :IMGS], trU[:80, :80], kqr[:80, :, 1],
                             start=False, stop=True)
            nc.vector.tensor_copy(rr[:80, :, 1], pr1[:80, :IMGS])
            for b in range(IMGS):
                ptb = prpool.tile([P, 2, P], F32, tag="ptb")
                for c in range(2):
                    # PT[s, j] = (j+1 == rank_s) * keep_s
                    nc.vector.tensor_scalar(
                        ptb[:, c, :], io1b[:, :], rr[:, b, c:c + 1],
                        kq[:, b, c:c + 1], OP.is_equal, OP.mult)
                pg = ps1.tile([P, 512], F32, tag="ps1")
                acc = pg[:, :C]
                nc.tensor.matmul(acc, ptb[:, 0, :], h[:, b, 0, :],
                                 start=True, stop=False)
                nc.tensor.matmul(acc, ptb[:80, 1, :], h[:80, b, 1, :],
                                 start=False, stop=True)
                nc.vector.tensor_copy(h[:, b, 0, :], acc)
            # rebuild key bias for the compacted layout (static):
            # 0 for partitions < NKEEP, NEG for the zero-padded tail
            nc.vector.memset(mb[:, :, 0], 0.0)
            nc.gpsimd.affine_select(
                out=mb[:, :, 0], in_=mb[:, :, 0], pattern=[[0, IMGS]],
                compare_op=OP.is_ge, fill=NEG, base=NKEEP - 1,
                channel_multiplier=-1)

        # ---------------- transformer layers ----------------
        # Software pipeline: LN (DVE stats + ACT normalize + PE transpose)
        # for compute-block k+2 is emitted before compute-block k, so the
        # LN chain of the next pair runs on DVE/ACT while the PE chews on
        # the current pair's matmuls.
        PW_MAX = 2 * W_A

        def geom_of(li):
            if li <= COMPACT_AT or NO_COMPACT:
                return CH_A, W_A, QWIN_A, QSH_A
            return CH_B, W_B, QWIN_B, QSH_B

        blocks = []
        for li in range(n_layers):
            for p in range(PAIRS):
                blocks.append(("A", li, p))
            for p in range(PAIRS):
                blocks.append(("M", li, p))

        xfms = {}
        stats_d = {}
        pend = {}

        def emit_stats_for(blk):
            kind, li, p = blk
            CHg, _, _, _ = geom_of(li)
            stats_d[blk] = ln_stats(p, CHg)

        def emit_ln_for(blk):
            kind, li, p = blk
            CHg, Wg, _, _ = geom_of(li)
            if blk not in stats_d:
                emit_stats_for(blk)
            rstd, nmean = stats_d.pop(blk)
            x = xpool.tile([P, 3, PW_MAX], F32R, tag="xfm")
            ln_apply(p, CHg, Wg, x, rstd, nmean)
            xfms[blk] = x

        def ln_blocked(cur, tgt):
            # LN of post-compaction blocks can't be emitted pre-compaction
            return (not NO_COMPACT) and tgt[1] > COMPACT_AT and cur[1] <= COMPACT_AT

        weights = {}

        def load_weights(li):
            wqk_t = wpool.tile([P, 3, 768], F32R, tag="wqk")
            nc.sync.dma_start(wqk_t[:], wqk.ap()[li].rearrange("k p m -> p k m"))
            bqk_t = wpool.tile([P, 6], F32, tag="bqk")
            nc.sync.dma_start(bqk_t[:], bqk.ap()[li].rearrange("m p -> p m"))
            wv_t = wpool1.tile([P, 3, 396], F32R, tag="wv")
            nc.sync.dma_start(wv_t[:], wv.ap()[li].rearrange("k p m -> p k m"))
            bv_t = wpool1.tile([1, 396], F32, tag="bv")
            nc.sync.dma_start(bv_t[:], bv.ap()[li])
            bv_m = wpool1.tile([P, 396], F32, tag="bvm")
            nc.gpsimd.partition_broadcast(bv_m[:], bv_t[:])
            wp_t = wpool1.tile([P, 3, C], F32R, tag="wp")
            nc.sync.dma_start(wp_t[:], wp.ap()[li].rearrange("k p m -> p k m"))
            bp_t = wpool1.tile([1, C], F32R, tag="bp")
            nc.sync.dma_start(bp_t[:], bp.ap()[li])
            bp_m = wpool1.tile([P, C], F32, tag="bpm")
            nc.gpsimd.partition_broadcast(bp_m[:], bp_t[:].bitcast(F32))
            w1_t = wpool1.tile([P, 3, MLP], F32R, tag="w1")
            for q4 in range(4):
                nc.sync.dma_start(
                    w1_t[:, :, q4 * 384:(q4 + 1) * 384],
                    w1.ap()[li].rearrange("k p m -> p k m")[:, :, q4 * 384:(q4 + 1) * 384])
            b1f_t = wpool1.tile([P, 12], F32, tag="b1")
            nc.sync.dma_start(b1f_t[:], b1.ap()[li].rearrange("o (m p) -> p (o m)", p=P))
            w2_t = wpool1.tile([P, 12, C], F16, tag="w2")
            for q4 in range(4):
                nc.sync.dma_start(
                    w2_t[:, q4 * 3:(q4 + 1) * 3, :],
                    w2.ap()[li].rearrange("k p m -> p k m")[:, q4 * 3:(q4 + 1) * 3, :])
            b2_t = wpool1.tile([1, C], F16, tag="b2")
            nc.sync.dma_start(b2_t[:], b2.ap()[li])
            b2_m = wpool1.tile([P, C], F16, tag="b2m")
            nc.gpsimd.partition_broadcast(b2_m[:], b2_t[:])
            weights[li] = (wqk_t, bqk_t, wv_t, bv_m, wp_t, bp_m,
                           w1_t, b1f_t, w2_t, b2_m)

        emit_stats_for(blocks[0])
        if len(blocks) > 1:
            emit_stats_for(blocks[1])
        emit_ln_for(blocks[0])
        if len(blocks) > 2:
            emit_stats_for(blocks[2])
        if len(blocks) > 1:
            emit_ln_for(blocks[1])
        for k, blk in enumerate(blocks):
            kind, li, p = blk
            CHg, Wg, QWINg, QSHg = geom_of(li)
            if blk not in xfms:          # deferred (post-compaction) LN
                emit_ln_for(blk)
            if kind == "A" and p == 0:
                load_weights(li)
            (wqk_t, bqk_t, wv_t, bv_m, wp_t, bp_t,
             w1_t, b1f_t, w2_t, b2_t) = weights[li]
            xfm = xfms.pop(blk)
            if kind == "A":
                pend[p] = attn_h1(li, p, CHg, Wg, QWINg, QSHg, xfm, wqk_t,
                                  bqk_t, wv_t, bv_m)
                if p > 0:
                    attn_h2(li, p - 1, CHg, Wg, QWINg, QSHg, pend.pop(p - 1),
                            wp_t, bp_t)
                if p == PAIRS - 1:
                    attn_h2(li, p, CHg, Wg, QWINg, QSHg, pend.pop(p),
                            wp_t, bp_t)
            else:
                mlp_pair(p, CHg, Wg, xfm, w1_t, b1f_t, w2_t, b2_t)
                if p == 0 and li in PRUNE:
                    mask_update(li)
                if (p == PAIRS - 1 and li == COMPACT_AT and not NO_COMPACT
                        and n_layers > COMPACT_AT + 1):
                    compact()
            if k + 3 < len(blocks) and not ln_blocked(blk, blocks[k + 3]):
                emit_stats_for(blocks[k + 3])
            if k + 2 < len(blocks) and not ln_blocked(blk, blocks[k + 2]):
                emit_ln_for(blocks[k + 2])

        # ---------------- final LN + head ----------------
        for b in range(IMGS):
            nc.sync.dma_start(cls_dram.ap()[b, :], h[0:1, b, 0, :])
        clst = prpool.tile([IMGS, C], F32, tag="clst")
        nc.sync.dma_start(clst[:], cls_dram.ap())
        s6 = prpool.tile([IMGS, 6], F32, tag="s6f")
        mv = prpool.tile([IMGS, 2], F32, tag="mvf")
        nc.vector.bn_stats(s6[:], clst[:])
        nc.vector.bn_aggr(mv[:], s6[:])
        rstd = prpool.tile([IMGS, 1], F32, tag="rstdf")
        _rsqrt(nc, spool, rstd, mv[:, 1:2], EPS)
        nmean = prpool.tile([IMGS, 1], F32, tag="nmeanf")
        nc.vector.scalar_tensor_tensor(nmean[:], mv[:, 0:1], -1.0, rstd[:],
                                       OP.mult, OP.mult)
        clsn = prpool.tile([IMGS, C], F32R, tag="clsn")
        nc.scalar.activation(clsn[:], clst[:], AF.Identity, bias=nmean[:],
                             scale=rstd[:])
        clsf = prpool.tile([P, 3, IMGS], F32R, tag="clsf")
        for f in range(3):
            pt = pstr.tile([P, 512], F32, tag="pstr")
            ptr = pt.bitcast(F32R)
            nc.tensor.transpose(ptr[:, :IMGS], clsn[:, f * P:(f + 1) * P],
                                ident[:IMGS, :IMGS])
            nc.vector.tensor_copy(clsf[:, f, :], ptr[:, :IMGS])
        wh_t = prpool.tile([P, 3, NCLS], F32R, tag="wht")
        nc.sync.dma_start(wh_t[:], wh.ap().rearrange("k p m -> p k m"))
        bh_t = prpool.tile([1, NCLS], F32R, tag="bht")
        nc.sync.dma_start(bh_t[:], bh.ap())
        po = ps1.tile([P, 512], F32, tag="ps1")
        acc = po[:IMGS, :NCLS]
        for kt in range(3):
            nc.tensor.matmul(acc, clsf[:, kt, :], wh_t[:, kt, :],
                             start=(kt == 0), stop=False)
        nc.tensor.matmul(acc, ones_r[:, :IMGS], bh_t[:], start=False, stop=True)
        ot = prpool.tile([IMGS, NCLS], F32, tag="outf")
        nc.vector.tensor_copy(ot[:], acc)
        nc.sync.dma_start(out.ap(), ot[:])

    nc.finalize()
    return nc


# ======================= host side =======================

def _prep(inputs):
    """Host-side: patchify x, fold LN affines, lay out weights."""
    f32 = np.float32
    f16 = np.float16
    d = {}
    x = np.asarray(inputs["x"], f32)
    Bn = x.shape[0]
    # patches feature-major, with token shift (col 0 = CLS placeholder)
    p = x.reshape(Bn, 3, 14, 16, 14, 16).transpose(0, 2, 4, 1, 3, 5)
    p = p.reshape(Bn, NPATCH, 768)
    xp = np.zeros((Bn, 768, W_A), f32)
    xp[:, :, 1:NTOK] = p.transpose(0, 2, 1)
    d["xp_all"] = xp.reshape(Bn, 6, 128, W_A)

    pw_ = np.asarray(inputs["patch_w"], f32)
    d["pw"] = pw_.reshape(6, 128, C)

    h0b = np.zeros((2, 128, C), f32)
    pos = np.asarray(inputs["pos_embed"], f32)[0]
    pb = np.asarray(inputs["patch_b"], f32)
    cls0 = np.asarray(inputs["cls_token"], f32).reshape(C) + pos[0]
    bias_tok = np.zeros((W_A, C), f32)
    bias_tok[0] = cls0
    bias_tok[1:NTOK] = pb[None, :] + pos[1:]
    for c, (off, wd) in enumerate(CH_A):
        h0b[c, :wd] = bias_tok[off:off + wd]
    d["h0b"] = h0b

    mb_ = np.zeros((2, 128), f32)
    for c, (off, wd) in enumerate(CH_A):
        for pp in range(128):
            t = off + pp
            if pp >= wd or t >= NTOK:
                mb_[c, pp] = NEG
    d["mb0"] = mb_

    qkv_w = np.asarray(inputs["qkv_w"], f32)
    qkv_b = np.asarray(inputs["qkv_b"], f32)
    g1 = np.asarray(inputs["ln1_g"], f32)
    b1_ = np.asarray(inputs["ln1_b"], f32)
    g2 = np.asarray(inputs["ln2_g"], f32)
    b2_ = np.asarray(inputs["ln2_b"], f32)

    wqk_l = np.zeros((LAYERS, 3, 128, 768), f32)
    bqk_l = np.zeros((LAYERS, 6, 128), f32)
    wv_l = np.zeros((LAYERS, 3, 128, 396), f32)
    bv_l = np.zeros((LAYERS, 1, 396), f32)
    for li in range(LAYERS):
        wq = qkv_w[li] * g1[li][:, None]          # [C, 3C] folded
        bq = qkv_b[li] + b1_[li] @ qkv_w[li]
        wqk2 = np.zeros((C, 768), f32)
        for m in range(6):
            wqk2[:, m * 128:m * 128 + 64] = wq[:, m * 64:(m + 1) * 64]
            wqk2[:, m * 128 + 64:m * 128 + 128] = \
                wq[:, 384 + m * 64:384 + (m + 1) * 64]
            bqk_l[li, m, 0:64] = bq[m * 64:(m + 1) * 64]
            bqk_l[li, m, 64:128] = bq[384 + m * 64:384 + (m + 1) * 64]
        wqk_l[li] = wqk2.reshape(3, 128, 768)
        wvl = np.zeros((C, 396), f32)
        bvl = np.zeros((396,), f32)
        for hh in range(HEADS):
            wvl[:, hh * 66:hh * 66 + 64] = wq[:, 768 + hh * 64:768 + (hh + 1) * 64]
            bvl[hh * 66:hh * 66 + 64] = bq[768 + hh * 64:768 + (hh + 1) * 64]
            bvl[hh * 66 + 64] = 1.0
        wv_l[li] = wvl.reshape(3, 128, 396)
        bv_l[li, 0] = bvl
    d["wqk"], d["bqk"], d["wv"], d["bv"] = wqk_l, bqk_l, wv_l, bv_l

    d["wp"] = np.asarray(inputs["proj_w"], f32).reshape(LAYERS, 3, 128, C)
    d["bp"] = np.asarray(inputs["proj_b"], f32).reshape(LAYERS, 1, C)
    w1_ = np.asarray(inputs["fc1_w"], f32) * g2[:, :, None]
    d["w1"] = w1_.reshape(LAYERS, 3, 128, MLP)
    d["b1"] = (np.asarray(inputs["fc1_b"], f32)
               + np.einsum('lc,lcm->lm', b2_, np.asarray(inputs["fc1_w"], f32))
               ).reshape(LAYERS, 1, MLP)
    d["w2"] = np.asarray(inputs["fc2_w"], f32).reshape(
        LAYERS, 12, 128, C).astype(f16)
    d["b2"] = np.asarray(inputs["fc2_b"], f32).reshape(
        LAYERS, 1, C).astype(f16)

    ng = np.asarray(inputs["norm_g"], f32)
    nb = np.asarray(inputs["norm_b"], f32)
    hw = np.asarray(inputs["head_w"], f32)
    d["wh"] = (hw * ng[:, None]).reshape(3, 128, NCLS)
    d["bh"] = (np.asarray(inputs["head_b"], f32) + nb @ hw).reshape(1, NCLS)
    return d


_NC_CACHE = {}


def kernel(**inputs):
    key = (N_LAYERS_BUILD, NO_COMPACT)
    if key not in _NC_CACHE:
        _NC_CACHE[key] = build_kernel()
    nc = _NC_CACHE[key]
    d = _prep(inputs)
    shared = {k: np.ascontiguousarray(v) for k, v in d.items() if k != "xp_all"}
    in_maps = []
    for core in range(8):
        m = dict(shared)
        m["xp"] = np.ascontiguousarray(
            d["xp_all"][core * IMGS:(core + 1) * IMGS].transpose(1, 2, 0, 3))
        in_maps.append(m)
    res = run_bass_kernel_spmd(nc, in_maps, core_ids=list(range(8)))
    outs = [r["out"] for r in res.results]
    return np.concatenate(outs, axis=0)


if __name__ == "__main__":
    print("building kernel ...")
    nc = build_kernel()
    print("built OK")


# revision 25
# speedup vs baseline: 1.1815x; 1.1815x over previous
"""AttentionPruneViT-Small Trainium2 kernel (Bass/Tile), data-parallel over
batch on 8 NeuronCores (8 images per core).

Self-contained: hardcodes all shapes; host side patchifies the input, folds
LN affines into adjacent weights, lays out weights for the device, runs the
Bass kernel on cores 0-7 and reassembles the [64, 100] output.

Numerics: fp32r matmuls (11-bit mantissa) everywhere except the attention
AV product (exp weights and V in fp16) and fc2 (gelu activations / weights
in fp16) -- validated on this input distribution. Softmax without
max-subtraction (scores are tiny). Token pruning is done by masking pruned
keys out of attention (exp bias of -1e30); after the last prune (layer 6)
the 120 surviving tokens are physically compacted into a single 128-token
chunk via a permutation matmul, so layers 7-11 run single-chunk.

Structure: layers are emitted per image-pair (LN stats/normalize for pair
p+1 overlap attention/MLP matmuls of pair p), keeping the PE dense so the
HAM clock gate stays at full rate.
"""
import os
import numpy as np
from contextlib import ExitStack

import concourse.bass as bass
import concourse.mybir as mybir
import concourse.tile as tile
from concourse import bacc
from concourse.bass_utils import run_bass_kernel_spmd

F32 = mybir.dt.float32
F32R = mybir.dt.float32r
BF16 = mybir.dt.bfloat16
F16 = mybir.dt.float16
AF = mybir.ActivationFunctionType
OP = mybir.AluOpType

# model constants
B = 64
C = 384
HEADS = 6
HD = 64
MLP = 1536
LAYERS = 12
NPATCH = 196
NTOK = 197
NCLS = 100
EPS = 1e-6
PRUNE = {2: 20, 4: 27, 6: 30}   # layer -> number of tokens dropped
SCALE = HD ** -0.5
NKEEP = 120                     # tokens surviving all prunes
COMPACT_AT = 6

# per-core geometry
IMGS = 8                 # images per core
PAIRS = IMGS // 2
NEG = -1e30

# geometry A: layers 0..6 (197 tokens, two chunks per image)
W_A = 208
CH_A = [(0, 128), (128, 80)]
QWIN_A = [0, 160]        # S-matmul rhs window start per image-in-pair
QSH_A = [0, 48]          # query column of token 0 within the window
# geometry B: layers 7..11 (120 tokens, one chunk per image)
W_B = 128
CH_B = [(0, 128)]
QWIN_B = [0, 0]
QSH_B = [0, 128]

N_LAYERS_BUILD = int(os.environ.get("VIT_LAYERS", str(LAYERS)))
NO_COMPACT = os.environ.get("VIT_NOCOMPACT", "") == "1"


def _rsqrt(nc, pool, out, var, eps):
    """out = 1/sqrt(var + eps): DVE add+reciprocal, then Sqrt on ACT."""
    P, n = var.shape[0], var.shape[1]
    iv = pool.tile([P, n], F32, tag="rsq_iv")
    nc.vector.tensor_scalar_add(iv[:], var[:], float(eps))
    nc.vector.reciprocal(iv[:], iv[:])
    nc.scalar.activation(out[:], iv[:], AF.Sqrt)


def build_kernel(n_layers=N_LAYERS_BUILD):
    nc = bacc.Bacc(target_bir_lowering=False)
    P = 128

    # ---------------- DRAM I/O ----------------
    xp = nc.dram_tensor("xp", [6, 128, IMGS, W_A], F32R, kind="ExternalInput")
    pw = nc.dram_tensor("pw", [6, 128, C], F32R, kind="ExternalInput")
    h0b = nc.dram_tensor("h0b", [2, 128, C], F32, kind="ExternalInput")
    mb0 = nc.dram_tensor("mb0", [2, 128], F32, kind="ExternalInput")
    wqk = nc.dram_tensor("wqk", [LAYERS, 3, 128, 768], F32R, kind="ExternalInput")
    bqk = nc.dram_tensor("bqk", [LAYERS, 6, 128], F32, kind="ExternalInput")
    wv = nc.dram_tensor("wv", [LAYERS, 3, 128, 396], F32R, kind="ExternalInput")
    bv = nc.dram_tensor("bv", [LAYERS, 1, 396], F32, kind="ExternalInput")
    wp = nc.dram_tensor("wp", [LAYERS, 3, 128, C], F32R, kind="ExternalInput")
    bp = nc.dram_tensor("bp", [LAYERS, 1, C], F32R, kind="ExternalInput")
    w1 = nc.dram_tensor("w1", [LAYERS, 3, 128, MLP], F32R, kind="ExternalInput")
    b1 = nc.dram_tensor("b1", [LAYERS, 1, MLP], F32, kind="ExternalInput")
    w2 = nc.dram_tensor("w2", [LAYERS, 12, 128, C], F16, kind="ExternalInput")
    b2 = nc.dram_tensor("b2", [LAYERS, 1, C], F16, kind="ExternalInput")
    wh = nc.dram_tensor("wh", [3, 128, NCLS], F32R, kind="ExternalInput")
    bh = nc.dram_tensor("bh", [1, NCLS], F32R, kind="ExternalInput")
    out = nc.dram_tensor("out", [IMGS, NCLS], F32, kind="ExternalOutput")
    # DRAM scratch for cross-partition bounces (prune bookkeeping)
    sc_dram = nc.dram_tensor("sc_dram", [IMGS, 2 * 128], F32)
    dm_dram = nc.dram_tensor("dm_dram", [IMGS, 2 * 128], F32)
    cls_dram = nc.dram_tensor("cls_dram", [IMGS, C], F32)

    with tile.TileContext(nc) as tc, ExitStack() as ctx:
        cpool = ctx.enter_context(tc.tile_pool(name="const", bufs=1))
        spool = ctx.enter_context(tc.tile_pool(name="stats", bufs=4))
        wpool = ctx.enter_context(tc.tile_pool(name="weights", bufs=2))
        wpool1 = ctx.enter_context(tc.tile_pool(name="weights1", bufs=1))
        apool = ctx.enter_context(tc.tile_pool(name="acts", bufs=2))
        qkpool = ctx.enter_context(tc.tile_pool(name="qkp", bufs=1))
        prpool = ctx.enter_context(tc.tile_pool(name="prp", bufs=1))
        vpool = ctx.enter_context(tc.tile_pool(name="vtile", bufs=6))
        ppool = ctx.enter_context(tc.tile_pool(name="ptile", bufs=8))
        opool = ctx.enter_context(tc.tile_pool(name="otile", bufs=2))
        xpool = ctx.enter_context(tc.tile_pool(name="xfm", bufs=3))
        gpool = ctx.enter_context(tc.tile_pool(name="gtile", bufs=1))
        ps1 = ctx.enter_context(tc.tile_pool(name="ps1", bufs=6, space="PSUM"))
        pstr = ctx.enter_context(tc.tile_pool(name="pstr", bufs=2, space="PSUM"))

        # persistent state
        h = cpool.tile([P, IMGS, 2, C], F32)              # residual stream
        mb = cpool.tile([P, IMGS, 2], F32)                # attention key bias
        ident = cpool.tile([P, P], F32R)
        ones_r = cpool.tile([1, P], F32R)
        ones_b = cpool.tile([1, P], F16)

        from concourse.masks import make_identity, make_upper_triangular
        identF = prpool.tile([P, P], F32, tag="identF")
        make_identity(nc, identF[:])
        nc.vector.tensor_copy(ident[:], identF[:])
        nc.vector.memset(h[:], 0.0)
        nc.vector.memset(ones_r[:].bitcast(F32), 1.0)
        nc.vector.memset(ones_b[:], 1.0)
        for b in range(IMGS):
            nc.sync.dma_start(mb[:, b, :], mb0.ap().rearrange("c p -> p c"))

        # compaction constants
        trU = cpool.tile([P, P], F32)       # trU[s, t] = 1 if s <= t
        make_upper_triangular(nc, trU[:], 1.0, diag=True)
        ones128 = cpool.tile([P, P], F32)
        nc.vector.memset(ones128[:], 1.0)
        io_row = cpool.tile([1, P], F32)
        nc.gpsimd.iota(io_row[:], [[1, P]], base=1, channel_multiplier=0,
                       allow_small_or_imprecise_dtypes=True)
        io1b = cpool.tile([P, P], F32)
        nc.gpsimd.partition_broadcast(io1b[:], io_row[:])

        h0b_t = prpool.tile([P, 2, C], F32, tag="h0b")
        nc.sync.dma_start(h0b_t[:], h0b.ap().rearrange("c p f -> p c f"))

        # ---------------- patch embed ----------------
        pw_t = cpool.tile([P, 6, C], F32R)
        nc.sync.dma_start(pw_t[:], pw.ap().rearrange("k p f -> p k f"))
        for b in range(IMGS):
            xp_t = gpool.tile([P, 6, W_A], F32R, tag="g")
            nc.sync.dma_start(xp_t[:], xp.ap()[:, :, b, :].rearrange("k p t -> p k t"))
            for c, (off, wd) in enumerate(CH_A):
                ps = ps1.tile([P, 512], F32, tag="ps1")
                acc = ps[:wd, :C]
                for kt in range(6):
                    nc.tensor.matmul(acc, xp_t[:, kt, off:off + wd],
                                     pw_t[:, kt, :], start=(kt == 0),
                                     stop=(kt == 5))
                nc.vector.tensor_tensor(h[:wd, b, c, :], acc,
                                        h0b_t[:wd, c, :], OP.add)

        # ---------------- per-layer helpers ----------------
        def ln_stats(p, CHg):
            """LN statistics for pair p (DVE only): returns (rstd, nmean)."""
            mv = spool.tile([P, 2, 2, 2], F32, tag="mv")
            nc.vector.memset(mv[:], 1.0)
            for bj in range(2):
                b = 2 * p + bj
                for c, (off, wd) in enumerate(CHg):
                    s6 = spool.tile([P, 6], F32, tag="s6")
                    nc.vector.bn_stats(s6[:wd, :], h[:wd, b, c, :])
                    nc.vector.bn_aggr(mv[:wd, bj, c, :], s6[:wd, :])
            rstd = spool.tile([P, 4], F32, tag="rstd")
            _rsqrt(nc, spool, rstd,
                   mv[:].rearrange("p b c s -> p (b c) s")[:, :, 1], EPS)
            nmean = spool.tile([P, 4], F32, tag="nmean")
            nc.vector.scalar_tensor_tensor(
                nmean[:], mv[:].rearrange("p b c s -> p (b c) s")[:, :, 0],
                -1.0, rstd[:], OP.mult, OP.mult)
            return rstd, nmean

        def ln_apply(p, CHg, Wg, xfm, rstd, nmean):
            """Normalize (ACT) + transpose (PE) into feature-major xfm."""
            for bj in range(2):
                b = 2 * p + bj
                for c, (off, wd) in enumerate(CHg):
                    xl = apool.tile([P, C], F32R, tag="xln")
                    i = bj * 2 + c
                    if i % 2 == 0:
                        nc.vector.tensor_scalar(
                            xl[:wd, :], h[:wd, b, c, :], rstd[:wd, i:i + 1],
                            nmean[:wd, i:i + 1], OP.mult, OP.add)
                    else:
                        nc.scalar.activation(
                            xl[:wd, :], h[:wd, b, c, :], AF.Identity,
                            bias=nmean[:wd, i:i + 1], scale=rstd[:wd, i:i + 1])
                    pt = pstr.tile([P, 512], F32, tag="pstr")
                    ptr = pt.bitcast(F32R)
                    for f in range(3):
                        nc.tensor.transpose(
                            ptr[:, f * P:f * P + wd], xl[:wd, f * P:(f + 1) * P],
                            ident[:wd, :wd])
                    srcv = ptr[:, :384].rearrange("p (f w) -> p f w", w=P)[:, :, :wd]
                    dst = xfm[:, :, bj * Wg + off:bj * Wg + off + wd]
                    if (bj * 2 + c) % 2 == 0:
                        nc.vector.tensor_copy(dst, srcv)
                    else:
                        nc.scalar.copy(dst, srcv)

        def attn_h1(li, p, CHg, Wg, QWINg, QSHg, xfm, wqk_t, bqk_t, wv_t,
                    bv_m):
            b0 = 2 * p
            PWg = 2 * Wg
            nch = len(CHg)
            # Q,K for the pair: per head one [128, PW] matmul (Q stacked
            # on partitions 0-63, K on 64-127), bias peeled off on ACT
            qt = qkpool.tile([64, 6, PW_MAX], F32R, tag="qt")
            kt_ = qkpool.tile([64, 6, PW_MAX], F32R, tag="kt")
            for m in range(6):
                pt = ps1.tile([P, 512], F32, tag="ps1")
                acc = pt[:, :PWg]
                for kt in range(3):
                    nc.tensor.matmul(acc, wqk_t[:, kt, m * 128:(m + 1) * 128],
                                     xfm[:, kt, :PWg],
                                     start=(kt == 0), stop=(kt == 2))
                nc.scalar.activation(qt[:, m, :PWg], acc[0:64, :], AF.Identity,
                                     bias=bqk_t[0:64, m:m + 1])
                nc.vector.tensor_scalar_add(kt_[:, m, :PWg], acc[64:128, :],
                                            bqk_t[64:128, m:m + 1])
            # V per image: token-major [tok, chunk, 6*66] bf16
            vts = []
            for j in (0, 1):
                vt = vpool.tile([P, 2, 396], F16, tag="vt")
                for c, (off, wd) in enumerate(CHg):
                    pv = ps1.tile([P, 512], F32, tag="ps1")
                    acc = pv[:wd, :396]
                    for kt in range(3):
                        nc.tensor.matmul(
                            acc, xfm[:, kt, j * Wg + off:j * Wg + off + wd],
                            wv_t[:, kt, :], start=(kt == 0), stop=(kt == 2))
                    nc.vector.tensor_tensor(vt[:wd, c, :], acc, bv_m[:wd, :],
                                            OP.add)
                vts.append(vt)
            # scores S^T + exp, image-interleaved; sps tiles are one PSUM
            # bank (2 heads) so S(next) overlaps exp(prev)
            ptss = [[None] * nch for _ in range(2)]
            e0s = [[None] * nch for _ in range(2)]
            for j in (0, 1):
                bi = b0 + j
                for c in range(nch):
                    ptss[j][c] = ppool.tile([P, 6, 256], F16, name="pt", tag="pt")
                    if li in PRUNE:
                        e0s[j][c] = prpool.tile([P, 6], F32, name=f"e0_{p % 2}{j}{c}", tag=f"e0_{p % 2}{j}{c}")
                for hp in range(3):
                    for c, (off, wd) in enumerate(CHg):
                        sps = ps1.tile([P, 2, 256], F32, name="sps", tag="ps1")
                        for hx in range(2):
                            hh = 2 * hp + hx
                            nc.tensor.matmul(
                                sps[:wd, hx, :],
                                kt_[:, hh, j * Wg + off:j * Wg + off + wd],
                                qt[:, hh, QWINg[j]:QWINg[j] + 256],
                                start=True, stop=True)
                        nc.scalar.activation(
                            ptss[j][c][:wd, 2 * hp:2 * hp + 2,
                                       QSHg[j]:QSHg[j] + Wg],
                            sps[:wd, :, QSHg[j]:QSHg[j] + Wg], AF.Exp,
                            bias=mb[:wd, bi, c:c + 1], scale=float(SCALE))
                        if li in PRUNE:
                            nc.scalar.activation(
                                e0s[j][c][:wd, 2 * hp:2 * hp + 2],
                                sps[:wd, :, QSHg[j]], AF.Exp,
                                bias=mb[:wd, bi, c:c + 1], scale=float(SCALE))
            return vts, ptss, e0s

        def attn_h2(li, p, CHg, Wg, QWINg, QSHg, state, wp_t, bp_t):
            b0 = 2 * p
            nch = len(CHg)
            vts, ptss, e0s = state
            # AV + denominators + proj, per image / query chunk
            for j in (0, 1):
                bi = b0 + j
                vt = vts[j]
                for qc, (qo, qw) in enumerate(CHg):
                    tps = ps1.tile([P, 512], F32, tag="ps1")
                    tview = tps[:, :396].rearrange("p (h c) -> p h c", c=66)
                    for hh in range(6):
                        for c, (off, wd) in enumerate(CHg):
                            nc.tensor.matmul(
                                tview[:qw, hh, :],
                                ptss[j][c][:wd, hh,
                                           QSHg[j] + qo:QSHg[j] + qo + qw],
                                vt[:wd, c, hh * 66:hh * 66 + 66],
                                start=(c == 0), stop=(c == nch - 1))
                    r = spool.tile([P, 6], F32, tag="rr")
                    nc.vector.reciprocal(r[:qw, :], tview[:qw, :, 64])
                    ot = opool.tile([P, 6, 64], F32R, tag="ot")
                    rap = r[:qw, :]
                    rb = bass.AP(rap.tensor, rap.offset,
                                 [list(x) for x in rap.ap] + [[0, 64]])
                    nc.vector.tensor_tensor(ot[:qw, :, :], tview[:qw, :, :64],
                                            rb, OP.mult)
                    # prune scores: weighted CLS column of exp tiles
                    if li in PRUNE and qc == 0:
                        wb = prpool.tile([P, 6], F32, tag="wb")
                        nc.gpsimd.partition_broadcast(wb[:], r[0:1, :])
                        sc = prpool.tile([P, 2], F32, tag="sc")
                        for c, (off, wd) in enumerate(CHg):
                            t6 = prpool.tile([P, 6], F32, tag="t6")
                            nc.vector.tensor_tensor(
                                t6[:wd, :], e0s[j][c][:wd, :], wb[:wd, :],
                                OP.mult)
                            nc.vector.reduce_sum(
                                sc[:wd, c:c + 1], t6[:wd, :],
                                axis=mybir.AxisListType.X)
                            nc.sync.dma_start(
                                sc_dram.ap()[bi, c * 128:c * 128 + wd],
                                sc[:wd, c])
                    # O^T -> feature-major via PE, then proj chunk
                    ofm = opool.tile([P, 3, P], F32R, tag="ofm")
                    pt2 = pstr.tile([P, 512], F32, tag="pstr")
                    pt2r = pt2.bitcast(F32R)
                    for f in range(3):
                        nc.tensor.transpose(
                            pt2r[:, f * P:f * P + qw],
                            ot[:qw, :, :].rearrange("p h d -> p (h d)")
                            [:, f * P:(f + 1) * P],
                            ident[:qw, :qw])
                    srco = pt2r[:, :384].rearrange("p (f w) -> p f w", w=P)[:, :, :qw]
                    if (j + qc) % 2 == 0:
                        nc.vector.tensor_copy(ofm[:, :, :qw], srco)
                    else:
                        nc.scalar.copy(ofm[:, :, :qw], srco)
                    pj = ps1.tile([P, 512], F32, tag="ps1")
                    acc = pj[:qw, :C]
                    for kt in range(3):
                        nc.tensor.matmul(acc, ofm[:, kt, :qw], wp_t[:, kt, :],
                                         start=(kt == 0), stop=(kt == 2))
                    yb = apool.tile([P, C], F32, tag="yb")
                    nc.vector.tensor_tensor(yb[:qw, :], acc, bp_t[:qw, :],
                                            OP.add)
                    nc.vector.tensor_tensor(h[:qw, bi, qc, :],
                                            h[:qw, bi, qc, :], yb[:qw, :],
                                            OP.add)

        def mlp_pair(p, CHg, Wg, xfm2, w1_t, b1f_t, w2_t, b2_t):
            b0 = 2 * p
            PWg = 2 * Wg
            g = gpool.tile([P, 12, PW_MAX], F16, tag="g")
            for m in range(12):
                f1 = ps1.tile([P, 512], F32, tag="ps1")
                acc = f1[:, :PWg]
                for kt in range(3):
                    nc.tensor.matmul(
                        acc, w1_t[:, kt, m * P:(m + 1) * P],
                        xfm2[:, kt, :PWg],
                        start=(kt == 0), stop=(kt == 2))
                nc.scalar.activation(g[:, m, :PWg], acc, AF.Gelu,
                                     bias=b1f_t[:, m:m + 1])
            for j in (0, 1):
                bi = b0 + j
                for c, (off, wd) in enumerate(CHg):
                    span = j * Wg + off
                    f2 = ps1.tile([P, 512], F32, tag="ps1")
                    acc = f2[:wd, :C]
                    for kt in range(12):
                        nc.tensor.matmul(acc, g[:, kt, span:span + wd],
                                         w2_t[:, kt, :],
                                         start=(kt == 0), stop=(kt == 11))
                    yb = apool.tile([P, C], F32, tag="yb")
                    nc.vector.tensor_tensor(yb[:wd, :], acc, b2_t[:wd, :],
                                            OP.add)
                    nc.vector.tensor_tensor(h[:wd, bi, c, :],
                                            h[:wd, bi, c, :], yb[:wd, :],
                                            OP.add)

        def mask_update(li):
            drop = PRUNE[li]
            scm = prpool.tile([IMGS, 256], F32, tag="scm")
            nc.sync.dma_start(scm[:], sc_dram.ap())
            # t = -1e9*(sc==0) - sc  over tokens 1..196
            tneg = prpool.tile([IMGS, 256], F32, tag="tneg")
            u = prpool.tile([IMGS, 256], F32, tag="uu")
            nc.vector.tensor_scalar(u[:, 1:NTOK], scm[:, 1:NTOK], 0.0, None,
                                    OP.is_equal)
            nc.vector.scalar_tensor_tensor(
                tneg[:, 1:NTOK], u[:, 1:NTOK], -1e9, scm[:, 1:NTOK],
                OP.mult, OP.subtract)
            m8 = prpool.tile([IMGS, 8], F32, tag="m8")
            left = drop
            while left > 0:
                k = min(8, left)
                nc.vector.max(m8[:], tneg[:, 1:NTOK])
                if k < 8:
                    nc.vector.memset(m8[:, k:], -2e30)
                nc.vector.match_replace(tneg[:, 1:NTOK], m8[:],
                                        tneg[:, 1:NTOK], NEG)
                left -= k
            dm = prpool.tile([IMGS, 256], F32, tag="dm")
            nc.vector.memset(dm[:], 0.0)
            nc.vector.tensor_scalar(dm[:, 1:NTOK], tneg[:, 1:NTOK], -1e29,
                                    None, OP.is_le)
            nc.sync.dma_start(dm_dram.ap(), dm[:])
            dmc = prpool.tile([P, IMGS, 2], F32, tag="dmc")
            for b in range(IMGS):
                nc.sync.dma_start(
                    dmc[:, b, :],
                    bass.AP(dm_dram, b * 256, [[1, 128], [128, 2]]))
            nc.vector.scalar_tensor_tensor(mb[:], dmc[:], NEG, mb[:],
                                           OP.mult, OP.add)

        def compact():
            """Gather the 120 kept tokens into chunk 0 (partitions 0..119)."""
            kq = prpool.tile([P, IMGS, 2], F32, tag="kq")
            nc.vector.tensor_scalar(kq[:], mb[:], 0.0, None, OP.is_equal)
            kqr = kq
            # ranks (inclusive prefix sums over token index) via PE
            rr = prpool.tile([P, IMGS, 2], F32, tag="rrk")
            pr0 = ps1.tile([P, 512], F32, tag="ps1")
            nc.tensor.matmul(pr0[:128, :IMGS], trU[:, :], kqr[:, :, 0],
                             start=True, stop=True)
            nc.vector.tensor_copy(rr[:, :, 0], pr0[:128, :IMGS])
            pr1 = ps1.tile([P, 512], F32, tag="ps1")
            nc.tensor.matmul(pr1[:80, :IMGS], ones128[:, :80], kqr[:, :, 0],
                             start=True, stop=False)
            nc.tensor.matmul(pr1[:80, :IMGS], trU[:80, :80], kqr[:80, :, 1],
                             start=False, stop=True)
            nc.vector.tensor_copy(rr[:80, :, 1], pr1[:80, :IMGS])
            for b in range(IMGS):
                ptb = prpool.tile([P, 2, P], F32, tag="ptb")
                for c in range(2):
                    # PT[s, j] = (j+1 == rank_s) * keep_s
                    nc.vector.tensor_scalar(
                        ptb[:, c, :], io1b[:, :], rr[:, b, c:c + 1],
                        kq[:, b, c:c + 1], OP.is_equal, OP.mult)
                pg = ps1.tile([P, 512], F32, tag="ps1")
                acc = pg[:, :C]
                nc.tensor.matmul(acc, ptb[:, 0, :], h[:, b, 0, :],
                                 start=True, stop=False)
                nc.tensor.matmul(acc, ptb[:80, 1, :], h[:80, b, 1, :],
                                 start=False, stop=True)
                nc.vector.tensor_copy(h[:, b, 0, :], acc)
            # rebuild key bias for the compacted layout (static):
            # 0 for partitions < NKEEP, NEG for the zero-padded tail
            nc.vector.memset(mb[:, :, 0], 0.0)
            nc.gpsimd.affine_select(
                out=mb[:, :, 0], in_=mb[:, :, 0], pattern=[[0, IMGS]],
                compare_op=OP.is_ge, fill=NEG, base=NKEEP - 1,
                channel_multiplier=-1)

        # ---------------- transformer layers ----------------
        # Software pipeline: LN (DVE stats + ACT normalize + PE transpose)
        # for compute-block k+2 is emitted before compute-block k, so the
        # LN chain of the next pair runs on DVE/ACT while the PE chews on
        # the current pair's matmuls.
        PW_MAX = 2 * W_A

        def geom_of(li):
            if li <= COMPACT_AT or NO_COMPACT:
                return CH_A, W_A, QWIN_A, QSH_A
            return CH_B, W_B, QWIN_B, QSH_B

        blocks = []
        for li in range(n_layers):
            for p in range(PAIRS):
                blocks.append(("A", li, p))
            for p in range(PAIRS):
                blocks.append(("M", li, p))

        xfms = {}
        stats_d = {}
        pend = {}

        def emit_stats_for(blk):
            kind, li, p = blk
            CHg, _, _, _ = geom_of(li)
            stats_d[blk] = ln_stats(p, CHg)

        def emit_ln_for(blk):
            kind, li, p = blk
            CHg, Wg, _, _ = geom_of(li)
            if blk not in stats_d:
                emit_stats_for(blk)
            rstd, nmean = stats_d.pop(blk)
            x = xpool.tile([P, 3, PW_MAX], F32R, tag="xfm")
            ln_apply(p, CHg, Wg, x, rstd, nmean)
            xfms[blk] = x

        def ln_blocked(cur, tgt):
            # LN of post-compaction blocks can't be emitted pre-compaction
            return (not NO_COMPACT) and tgt[1] > COMPACT_AT and cur[1] <= COMPACT_AT

        weights = {}

        def load_weights(li):
            wqk_t = wpool.tile([P, 3, 768], F32R, tag="wqk")
            nc.sync.dma_start(wqk_t[:], wqk.ap()[li].rearrange("k p m -> p k m"))
            bqk_t = wpool.tile([P, 6], F32, tag="bqk")
            nc.sync.dma_start(bqk_t[:], bqk.ap()[li].rearrange("m p -> p m"))
            wv_t = wpool1.tile([P, 3, 396], F32R, tag="wv")
            nc.sync.dma_start(wv_t[:], wv.ap()[li].rearrange("k p m -> p k m"))
            bv_t = wpool1.tile([1, 396], F32, tag="bv")
            nc.sync.dma_start(bv_t[:], bv.ap()[li])
            bv_m = wpool1.tile([P, 396], F32, tag="bvm")
            nc.gpsimd.partition_broadcast(bv_m[:], bv_t[:])
            wp_t = wpool1.tile([P, 3, C], F32R, tag="wp")
            nc.sync.dma_start(wp_t[:], wp.ap()[li].rearrange("k p m -> p k m"))
            bp_t = wpool1.tile([1, C], F32R, tag="bp")
            nc.sync.dma_start(bp_t[:], bp.ap()[li])
            bp_m = wpool1.tile([P, C], F32, tag="bpm")
            nc.gpsimd.partition_broadcast(bp_m[:], bp_t[:].bitcast(F32))
            w1_t = wpool1.tile([P, 3, MLP], F32R, tag="w1")
            for q4 in range(4):
                nc.sync.dma_start(
                    w1_t[:, :, q4 * 384:(q4 + 1) * 384],
                    w1.ap()[li].rearrange("k p m -> p k m")[:, :, q4 * 384:(q4 + 1) * 384])
            b1f_t = wpool1.tile([P, 12], F32, tag="b1")
            nc.sync.dma_start(b1f_t[:], b1.ap()[li].rearrange("o (m p) -> p (o m)", p=P))
            w2_t = wpool1.tile([P, 12, C], F16, tag="w2")
            for q4 in range(4):
                nc.sync.dma_start(
                    w2_t[:, q4 * 3:(q4 + 1) * 3, :],
                    w2.ap()[li].rearrange("k p m -> p k m")[:, q4 * 3:(q4 + 1) * 3, :])
            b2_t = wpool1.tile([1, C], F16, tag="b2")
            nc.sync.dma_start(b2_t[:], b2.ap()[li])
            b2_m = wpool1.tile([P, C], F16, tag="b2m")
            nc.gpsimd.partition_broadcast(b2_m[:], b2_t[:])
            weights[li] = (wqk_t, bqk_t, wv_t, bv_m, wp_t, bp_m,
                           w1_t, b1f_t, w2_t, b2_m)

        emit_stats_for(blocks[0])
        if len(blocks) > 1:
            emit_stats_for(blocks[1])
        emit_ln_for(blocks[0])
        if len(blocks) > 2:
            emit_stats_for(blocks[2])
        if len(blocks) > 1:
            emit_ln_for(blocks[1])
        for k, blk in enumerate(blocks):
            kind, li, p = blk
            CHg, Wg, QWINg, QSHg = geom_of(li)
            if blk not in xfms:          # deferred (post-compaction) LN
                emit_ln_for(blk)
            if kind == "A" and p == 0:
                load_weights(li)
            (wqk_t, bqk_t, wv_t, bv_m, wp_t, bp_t,
             w1_t, b1f_t, w2_t, b2_t) = weights[li]
            xfm = xfms.pop(blk)
            if kind == "A":
                pend[p] = attn_h1(li, p, CHg, Wg, QWINg, QSHg, xfm, wqk_t,
                                  bqk_t, wv_t, bv_m)
                if p > 0:
                    attn_h2(li, p - 1, CHg, Wg, QWINg, QSHg, pend.pop(p - 1),
                            wp_t, bp_t)
                if p == PAIRS - 1:
                    attn_h2(li, p, CHg, Wg, QWINg, QSHg, pend.pop(p),
                            wp_t, bp_t)
            else:
                mlp_pair(p, CHg, Wg, xfm, w1_t, b1f_t, w2_t, b2_t)
                if p == 0 and li in PRUNE:
                    mask_update(li)
                if (p == PAIRS - 1 and li == COMPACT_AT and not NO_COMPACT
                        and n_layers > COMPACT_AT + 1):
                    compact()
            if k + 3 < len(blocks) and not ln_blocked(blk, blocks[k + 3]):
                emit_stats_for(blocks[k + 3])
            if k + 2 < len(blocks) and not ln_blocked(blk, blocks[k + 2]):
                emit_ln_for(blocks[k + 2])

        # ---------------- final LN + head ----------------
        for b in range(IMGS):
            nc.sync.dma_start(cls_dram.ap()[b, :], h[0:1, b, 0, :])
        clst = prpool.tile([IMGS, C], F32, tag="clst")
        nc.sync.dma_start(clst[:], cls_dram.ap())
        s6 = prpool.tile([IMGS, 6], F32, tag="s6f")
        mv = prpool.tile([IMGS, 2], F32, tag="mvf")
        nc.vector.bn_stats(s6[:], clst[:])
        nc.vector.bn_aggr(mv[:], s6[:])
        rstd = prpool.tile([IMGS, 1], F32, tag="rstdf")
        _rsqrt(nc, spool, rstd, mv[:, 1:2], EPS)
        nmean = prpool.tile([IMGS, 1], F32, tag="nmeanf")
        nc.vector.scalar_tensor_tensor(nmean[:], mv[:, 0:1], -1.0, rstd[:],
                                       OP.mult, OP.mult)
        clsn = prpool.tile([IMGS, C], F32R, tag="clsn")
        nc.scalar.activation(clsn[:], clst[:], AF.Identity, bias=nmean[:],
                             scale=rstd[:])
        clsf = prpool.tile([P, 3, IMGS], F32R, tag="clsf")
        for f in range(3):
            pt = pstr.tile([P, 512], F32, tag="pstr")
            ptr = pt.bitcast(F32R)
            nc.tensor.transpose(ptr[:, :IMGS], clsn[:, f * P:(f + 1) * P],
                                ident[:IMGS, :IMGS])
            nc.vector.tensor_copy(clsf[:, f, :], ptr[:, :IMGS])
        wh_t = prpool.tile([P, 3, NCLS], F32R, tag="wht")
        nc.sync.dma_start(wh_t[:], wh.ap().rearrange("k p m -> p k m"))
        bh_t = prpool.tile([1, NCLS], F32R, tag="bht")
        nc.sync.dma_start(bh_t[:], bh.ap())
        po = ps1.tile([P, 512], F32, tag="ps1")
        acc = po[:IMGS, :NCLS]
        for kt in range(3):
            nc.tensor.matmul(acc, clsf[:, kt, :], wh_t[:, kt, :],
                             start=(kt == 0), stop=False)
        nc.tensor.matmul(acc, ones_r[:, :IMGS], bh_t[:], start=False, stop=True)
        ot = prpool.tile([IMGS, NCLS], F32, tag="outf")
        nc.vector.tensor_copy(ot[:], acc)
        nc.sync.dma_start(out.ap(), ot[:])

    nc.finalize()
    return nc


# ======================= host side =======================

def _prep(inputs):
    """Host-side: patchify x, fold LN affines, lay out weights."""
    f32 = np.float32
    f16 = np.float16
    d = {}
    x = np.asarray(inputs["x"], f32)
    Bn = x.shape[0]
    # patches feature-major, with token shift (col 0 = CLS placeholder)
    p = x.reshape(Bn, 3, 14, 16, 14, 16).transpose(0, 2, 4, 1, 3, 5)
    p = p.reshape(Bn, NPATCH, 768)
    xp = np.zeros((Bn, 768, W_A), f32)
    xp[:, :, 1:NTOK] = p.transpose(0, 2, 1)
    d["xp_all"] = xp.reshape(Bn, 6, 128, W_A)

    pw_ = np.asarray(inputs["patch_w"], f32)
    d["pw"] = pw_.reshape(6, 128, C)

    h0b = np.zeros((2, 128, C), f32)
    pos = np.asarray(inputs["pos_embed"], f32)[0]
    pb = np.asarray(inputs["patch_b"], f32)
    cls0 = np.asarray(inputs["cls_token"], f32).reshape(C) + pos[0]
    bias_tok = np.zeros((W_A, C), f32)
    bias_tok[0] = cls0
    bias_tok[1:NTOK] = pb[None, :] + pos[1:]
    for c, (off, wd) in enumerate(CH_A):
        h0b[c, :wd] = bias_tok[off:off + wd]
    d["h0b"] = h0b

    mb_ = np.zeros((2, 128), f32)
    for c, (off, wd) in enumerate(CH_A):
        for pp in range(128):
            t = off + pp
            if pp >= wd or t >= NTOK:
                mb_[c, pp] = NEG
    d["mb0"] = mb_

    qkv_w = np.asarray(inputs["qkv_w"], f32)
    qkv_b = np.asarray(inputs["qkv_b"], f32)
    g1 = np.asarray(inputs["ln1_g"], f32)
    b1_ = np.asarray(inputs["ln1_b"], f32)
    g2 = np.asarray(inputs["ln2_g"], f32)
    b2_ = np.asarray(inputs["ln2_b"], f32)

    wqk_l = np.zeros((LAYERS, 3, 128, 768), f32)
    bqk_l = np.zeros((LAYERS, 6, 128), f32)
    wv_l = np.zeros((LAYERS, 3, 128, 396), f32)
    bv_l = np.zeros((LAYERS, 1, 396), f32)
    for li in range(LAYERS):
        wq = qkv_w[li] * g1[li][:, None]          # [C, 3C] folded
        bq = qkv_b[li] + b1_[li] @ qkv_w[li]
        wqk2 = np.zeros((C, 768), f32)
        for m in range(6):
            wqk2[:, m * 128:m * 128 + 64] = wq[:, m * 64:(m + 1) * 64]
            wqk2[:, m * 128 + 64:m * 128 + 128] = \
                wq[:, 384 + m * 64:384 + (m + 1) * 64]
            bqk_l[li, m, 0:64] = bq[m * 64:(m + 1) * 64]
            bqk_l[li, m, 64:128] = bq[384 + m * 64:384 + (m + 1) * 64]
        wqk_l[li] = wqk2.reshape(3, 128, 768)
        wvl = np.zeros((C, 396), f32)
        bvl = np.zeros((396,), f32)
        for hh in range(HEADS):
            wvl[:, hh * 66:hh * 66 + 64] = wq[:, 768 + hh * 64:768 + (hh + 1) * 64]
            bvl[hh * 66:hh * 66 + 64] = bq[768 + hh * 64:768 + (hh + 1) * 64]
            bvl[hh * 66 + 64] = 1.0
        wv_l[li] = wvl.reshape(3, 128, 396)
        bv_l[li, 0] = bvl
    d["wqk"], d["bqk"], d["wv"], d["bv"] = wqk_l, bqk_l, wv_l, bv_l

    d["wp"] = np.asarray(inputs["proj_w"], f32).reshape(LAYERS, 3, 128, C)
    d["bp"] = np.asarray(inputs["proj_b"], f32).reshape(LAYERS, 1, C)
    w1_ = np.asarray(inputs["fc1_w"], f32) * g2[:, :, None]
    d["w1"] = w1_.reshape(LAYERS, 3, 128, MLP)
    d["b1"] = (np.asarray(inputs["fc1_b"], f32)
               + np.einsum('lc,lcm->lm', b2_, np.asarray(inputs["fc1_w"], f32))
               ).reshape(LAYERS, 1, MLP)
    d["w2"] = np.asarray(inputs["fc2_w"], f32).reshape(
        LAYERS, 12, 128, C).astype(f16)
    d["b2"] = np.asarray(inputs["fc2_b"], f32).reshape(
        LAYERS, 1, C).astype(f16)

    ng = np.asarray(inputs["norm_g"], f32)
    nb = np.asarray(inputs["norm_b"], f32)
    hw = np.asarray(inputs["head_w"], f32)
    d["wh"] = (hw * ng[:, None]).reshape(3, 128, NCLS)
    d["bh"] = (np.asarray(inputs["head_b"], f32) + nb @ hw).reshape(1, NCLS)
    return d


_NC_CACHE = {}


def kernel(**inputs):
    key = (N_LAYERS_BUILD, NO_COMPACT)
    if key not in _NC_CACHE:
        _NC_CACHE[key] = build_kernel()
    nc = _NC_CACHE[key]
    d = _prep(inputs)
    shared = {k: np.ascontiguousarray(v) for k, v in d.items() if k != "xp_all"}
    in_maps = []
    for core in range(8):
        m = dict(shared)
        m["xp"] = np.ascontiguousarray(
            d["xp_all"][core * IMGS:(core + 1) * IMGS].transpose(1, 2, 0, 3))
        in_maps.append(m)
    res = run_bass_kernel_spmd(nc, in_maps, core_ids=list(range(8)))
    outs = [r["out"] for r in res.results]
    return np.concatenate(outs, axis=0)


if __name__ == "__main__":
    print("building kernel ...")
    nc = build_kernel()
    print("built OK")


# revision 26
# speedup vs baseline: 1.1849x; 1.0029x over previous
"""AttentionPruneViT-Small Trainium2 kernel (Bass/Tile), data-parallel over
batch on 8 NeuronCores (8 images per core).

Self-contained: hardcodes all shapes; host side patchifies the input, folds
LN affines into adjacent weights, lays out weights for the device, runs the
Bass kernel on cores 0-7 and reassembles the [64, 100] output.

Numerics: fp32r matmuls (11-bit mantissa) everywhere except the attention
AV product (exp weights and V in fp16) and fc2 (gelu activations / weights
in fp16) -- validated on this input distribution. Softmax without
max-subtraction (scores are tiny). Token pruning is done by masking pruned
keys out of attention (exp bias of -1e30); after the last prune (layer 6)
the 120 surviving tokens are physically compacted into a single 128-token
chunk via a permutation matmul, so layers 7-11 run single-chunk.

Structure: layers are emitted per image-pair (LN stats/normalize for pair
p+1 overlap attention/MLP matmuls of pair p), keeping the PE dense so the
HAM clock gate stays at full rate.
"""
import os
import numpy as np
from contextlib import ExitStack

import concourse.bass as bass
import concourse.mybir as mybir
import concourse.tile as tile
from concourse import bacc
from concourse.bass_utils import run_bass_kernel_spmd

F32 = mybir.dt.float32
F32R = mybir.dt.float32r
BF16 = mybir.dt.bfloat16
F16 = mybir.dt.float16
AF = mybir.ActivationFunctionType
OP = mybir.AluOpType

# model constants
B = 64
C = 384
HEADS = 6
HD = 64
MLP = 1536
LAYERS = 12
NPATCH = 196
NTOK = 197
NCLS = 100
EPS = 1e-6
PRUNE = {2: 20, 4: 27, 6: 30}   # layer -> number of tokens dropped
SCALE = HD ** -0.5
NKEEP = 120                     # tokens surviving all prunes
COMPACT_AT = 6

# per-core geometry
IMGS = 8                 # images per core
PAIRS = IMGS // 2
NEG = -1e30

# geometry A: layers 0..6 (197 tokens, two chunks per image)
W_A = 208
CH_A = [(0, 128), (128, 80)]
QWIN_A = [0, 160]        # S-matmul rhs window start per image-in-pair
QSH_A = [0, 48]          # query column of token 0 within the window
# geometry B: layers 7..11 (120 tokens, one chunk per image)
W_B = 128
CH_B = [(0, 128)]
QWIN_B = [0, 0]
QSH_B = [0, 128]

N_LAYERS_BUILD = int(os.environ.get("VIT_LAYERS", str(LAYERS)))
NO_COMPACT = os.environ.get("VIT_NOCOMPACT", "") == "1"


def _rsqrt(nc, pool, out, var, eps):
    """out = 1/sqrt(var + eps): DVE add+reciprocal, then Sqrt on ACT."""
    P, n = var.shape[0], var.shape[1]
    iv = pool.tile([P, n], F32, tag="rsq_iv")
    nc.vector.tensor_scalar_add(iv[:], var[:], float(eps))
    nc.vector.reciprocal(iv[:], iv[:])
    nc.scalar.activation(out[:], iv[:], AF.Sqrt)


def build_kernel(n_layers=N_LAYERS_BUILD):
    nc = bacc.Bacc(target_bir_lowering=False)
    P = 128

    # ---------------- DRAM I/O ----------------
    xp = nc.dram_tensor("xp", [6, 128, IMGS, W_A], F32R, kind="ExternalInput")
    pw = nc.dram_tensor("pw", [6, 128, C], F32R, kind="ExternalInput")
    h0b = nc.dram_tensor("h0b", [2, 128, C], F32, kind="ExternalInput")
    mb0 = nc.dram_tensor("mb0", [2, 128], F32, kind="ExternalInput")
    wqk = nc.dram_tensor("wqk", [LAYERS, 3, 128, 768], F32R, kind="ExternalInput")
    bqk = nc.dram_tensor("bqk", [LAYERS, 6, 128], F32, kind="ExternalInput")
    wv = nc.dram_tensor("wv", [LAYERS, 3, 128, 396], F32R, kind="ExternalInput")
    bv = nc.dram_tensor("bv", [LAYERS, 1, 396], F32, kind="ExternalInput")
    wp = nc.dram_tensor("wp", [LAYERS, 3, 128, C], F32R, kind="ExternalInput")
    bp = nc.dram_tensor("bp", [LAYERS, 1, C], F32R, kind="ExternalInput")
    w1 = nc.dram_tensor("w1", [LAYERS, 3, 128, MLP], F32R, kind="ExternalInput")
    b1 = nc.dram_tensor("b1", [LAYERS, 1, MLP], F32, kind="ExternalInput")
    w2 = nc.dram_tensor("w2", [LAYERS, 12, 128, C], F16, kind="ExternalInput")
    b2 = nc.dram_tensor("b2", [LAYERS, 1, C], F16, kind="ExternalInput")
    wh = nc.dram_tensor("wh", [3, 128, NCLS], F32R, kind="ExternalInput")
    bh = nc.dram_tensor("bh", [1, NCLS], F32R, kind="ExternalInput")
    out = nc.dram_tensor("out", [IMGS, NCLS], F32, kind="ExternalOutput")
    # DRAM scratch for cross-partition bounces (prune bookkeeping)
    sc_dram = nc.dram_tensor("sc_dram", [IMGS, 2 * 128], F32)
    dm_dram = nc.dram_tensor("dm_dram", [IMGS, 2 * 128], F32)
    cls_dram = nc.dram_tensor("cls_dram", [IMGS, C], F32)

    with tile.TileContext(nc) as tc, ExitStack() as ctx:
        cpool = ctx.enter_context(tc.tile_pool(name="const", bufs=1))
        spool = ctx.enter_context(tc.tile_pool(name="stats", bufs=4))
        wpool = ctx.enter_context(tc.tile_pool(name="weights", bufs=2))
        wpool1 = ctx.enter_context(tc.tile_pool(name="weights1", bufs=1))
        apool = ctx.enter_context(tc.tile_pool(name="acts", bufs=2))
        qkpool = ctx.enter_context(tc.tile_pool(name="qkp", bufs=1))
        prpool = ctx.enter_context(tc.tile_pool(name="prp", bufs=1))
        vpool = ctx.enter_context(tc.tile_pool(name="vtile", bufs=6))
        ppool = ctx.enter_context(tc.tile_pool(name="ptile", bufs=8))
        opool = ctx.enter_context(tc.tile_pool(name="otile", bufs=2))
        xpool = ctx.enter_context(tc.tile_pool(name="xfm", bufs=3))
        gpool = ctx.enter_context(tc.tile_pool(name="gtile", bufs=1))
        ps1 = ctx.enter_context(tc.tile_pool(name="ps1", bufs=6, space="PSUM"))
        pstr = ctx.enter_context(tc.tile_pool(name="pstr", bufs=2, space="PSUM"))

        # persistent state
        h = cpool.tile([P, IMGS, 2, C], F32)              # residual stream
        mb = cpool.tile([P, IMGS, 2], F32)                # attention key bias
        ident = cpool.tile([P, P], F32R)
        ones_r = cpool.tile([1, P], F32R)
        ones_b = cpool.tile([1, P], F16)

        from concourse.masks import make_identity, make_upper_triangular
        identF = prpool.tile([P, P], F32, tag="identF")
        make_identity(nc, identF[:])
        nc.vector.tensor_copy(ident[:], identF[:])
        nc.vector.memset(h[:], 0.0)
        nc.vector.memset(ones_r[:].bitcast(F32), 1.0)
        nc.vector.memset(ones_b[:], 1.0)
        for b in range(IMGS):
            nc.sync.dma_start(mb[:, b, :], mb0.ap().rearrange("c p -> p c"))

        # compaction constants
        trU = cpool.tile([P, P], F32)       # trU[s, t] = 1 if s <= t
        make_upper_triangular(nc, trU[:], 1.0, diag=True)
        ones128 = cpool.tile([P, P], F32)
        nc.vector.memset(ones128[:], 1.0)
        io_row = cpool.tile([1, P], F32)
        nc.gpsimd.iota(io_row[:], [[1, P]], base=1, channel_multiplier=0,
                       allow_small_or_imprecise_dtypes=True)
        io1b = cpool.tile([P, P], F32)
        nc.gpsimd.partition_broadcast(io1b[:], io_row[:])

        h0b_t = prpool.tile([P, 2, C], F32, tag="h0b")
        nc.sync.dma_start(h0b_t[:], h0b.ap().rearrange("c p f -> p c f"))

        # ---------------- patch embed ----------------
        pw_t = cpool.tile([P, 6, C], F32R)
        nc.sync.dma_start(pw_t[:], pw.ap().rearrange("k p f -> p k f"))
        for b in range(IMGS):
            xp_t = gpool.tile([P, 6, W_A], F32R, tag="g")
            nc.sync.dma_start(xp_t[:], xp.ap()[:, :, b, :].rearrange("k p t -> p k t"))
            for c, (off, wd) in enumerate(CH_A):
                ps = ps1.tile([P, 512], F32, tag="ps1")
                acc = ps[:wd, :C]
                for kt in range(6):
                    nc.tensor.matmul(acc, xp_t[:, kt, off:off + wd],
                                     pw_t[:, kt, :], start=(kt == 0),
                                     stop=(kt == 5))
                nc.vector.tensor_tensor(h[:wd, b, c, :], acc,
                                        h0b_t[:wd, c, :], OP.add)

        # ---------------- per-layer helpers ----------------
        def ln_stats(p, CHg):
            """LN statistics for pair p (DVE only): returns (rstd, nmean)."""
            mv = spool.tile([P, 2, 2, 2], F32, tag="mv")
            nc.vector.memset(mv[:], 1.0)
            for bj in range(2):
                b = 2 * p + bj
                for c, (off, wd) in enumerate(CHg):
                    s6 = spool.tile([P, 6], F32, tag="s6")
                    nc.vector.bn_stats(s6[:wd, :], h[:wd, b, c, :])
                    nc.vector.bn_aggr(mv[:wd, bj, c, :], s6[:wd, :])
            rstd = spool.tile([P, 4], F32, tag="rstd")
            _rsqrt(nc, spool, rstd,
                   mv[:].rearrange("p b c s -> p (b c) s")[:, :, 1], EPS)
            nmean = spool.tile([P, 4], F32, tag="nmean")
            nc.vector.scalar_tensor_tensor(
                nmean[:], mv[:].rearrange("p b c s -> p (b c) s")[:, :, 0],
                -1.0, rstd[:], OP.mult, OP.mult)
            return rstd, nmean

        def ln_apply(p, CHg, Wg, xfm, rstd, nmean, to_dve=False):
            """Normalize (ACT) + transpose (PE) into feature-major xfm."""
            for bj in range(2):
                b = 2 * p + bj
                for c, (off, wd) in enumerate(CHg):
                    xl = apool.tile([P, C], F32R, tag="xln")
                    i = bj * 2 + c
                    if i % 2 == 0:
                        nc.vector.tensor_scalar(
                            xl[:wd, :], h[:wd, b, c, :], rstd[:wd, i:i + 1],
                            nmean[:wd, i:i + 1], OP.mult, OP.add)
                    else:
                        nc.scalar.activation(
                            xl[:wd, :], h[:wd, b, c, :], AF.Identity,
                            bias=nmean[:wd, i:i + 1], scale=rstd[:wd, i:i + 1])
                    pt = pstr.tile([P, 512], F32, tag="pstr")
                    ptr = pt.bitcast(F32R)
                    for f in range(3):
                        nc.tensor.transpose(
                            ptr[:, f * P:f * P + wd], xl[:wd, f * P:(f + 1) * P],
                            ident[:wd, :wd])
                    srcv = ptr[:, :384].rearrange("p (f w) -> p f w", w=P)[:, :, :wd]
                    dst = xfm[:, :, bj * Wg + off:bj * Wg + off + wd]
                    if to_dve or (bj * 2 + c) % 2 == 0:
                        nc.vector.tensor_copy(dst, srcv)
                    else:
                        nc.scalar.copy(dst, srcv)

        def attn_h1(li, p, CHg, Wg, QWINg, QSHg, xfm, wqk_t, bqk_t, wv_t,
                    bv_m):
            b0 = 2 * p
            PWg = 2 * Wg
            nch = len(CHg)
            # Q,K for the pair: per head one [128, PW] matmul (Q stacked
            # on partitions 0-63, K on 64-127), bias peeled off on ACT
            qt = qkpool.tile([64, 6, PW_MAX], F32R, tag="qt")
            kt_ = qkpool.tile([64, 6, PW_MAX], F32R, tag="kt")
            for m in range(6):
                pt = ps1.tile([P, 512], F32, tag="ps1")
                acc = pt[:, :PWg]
                for kt in range(3):
                    nc.tensor.matmul(acc, wqk_t[:, kt, m * 128:(m + 1) * 128],
                                     xfm[:, kt, :PWg],
                                     start=(kt == 0), stop=(kt == 2))
                nc.scalar.activation(qt[:, m, :PWg], acc[0:64, :], AF.Identity,
                                     bias=bqk_t[0:64, m:m + 1])
                nc.vector.tensor_scalar_add(kt_[:, m, :PWg], acc[64:128, :],
                                            bqk_t[64:128, m:m + 1])
            # V per image: token-major [tok, chunk, 6*66] bf16
            vts = []
            for j in (0, 1):
                vt = vpool.tile([P, 2, 396], F16, tag="vt")
                for c, (off, wd) in enumerate(CHg):
                    pv = ps1.tile([P, 512], F32, tag="ps1")
                    acc = pv[:wd, :396]
                    for kt in range(3):
                        nc.tensor.matmul(
                            acc, xfm[:, kt, j * Wg + off:j * Wg + off + wd],
                            wv_t[:, kt, :], start=(kt == 0), stop=(kt == 2))
                    nc.vector.tensor_tensor(vt[:wd, c, :], acc, bv_m[:wd, :],
                                            OP.add)
                vts.append(vt)
            # scores S^T + exp, image-interleaved; sps tiles are one PSUM
            # bank (2 heads) so S(next) overlaps exp(prev)
            ptss = [[None] * nch for _ in range(2)]
            e0s = [[None] * nch for _ in range(2)]
            for j in (0, 1):
                bi = b0 + j
                for c in range(nch):
                    ptss[j][c] = ppool.tile([P, 6, 256], F16, name="pt", tag="pt")
                    if li in PRUNE:
                        e0s[j][c] = prpool.tile([P, 6], F32, name=f"e0_{p % 2}{j}{c}", tag=f"e0_{p % 2}{j}{c}")
                for hp in range(3):
                    for c, (off, wd) in enumerate(CHg):
                        sps = ps1.tile([P, 2, 256], F32, name="sps", tag="ps1")
                        for hx in range(2):
                            hh = 2 * hp + hx
                            nc.tensor.matmul(
                                sps[:wd, hx, :],
                                kt_[:, hh, j * Wg + off:j * Wg + off + wd],
                                qt[:, hh, QWINg[j]:QWINg[j] + 256],
                                start=True, stop=True)
                        nc.scalar.activation(
                            ptss[j][c][:wd, 2 * hp:2 * hp + 2,
                                       QSHg[j]:QSHg[j] + Wg],
                            sps[:wd, :, QSHg[j]:QSHg[j] + Wg], AF.Exp,
                            bias=mb[:wd, bi, c:c + 1], scale=float(SCALE))
                        if li in PRUNE:
                            nc.scalar.activation(
                                e0s[j][c][:wd, 2 * hp:2 * hp + 2],
                                sps[:wd, :, QSHg[j]], AF.Exp,
                                bias=mb[:wd, bi, c:c + 1], scale=float(SCALE))
            return vts, ptss, e0s

        def attn_h2(li, p, CHg, Wg, QWINg, QSHg, state, wp_t, bp_t):
            b0 = 2 * p
            nch = len(CHg)
            vts, ptss, e0s = state
            # AV + denominators + proj, per image / query chunk
            for j in (0, 1):
                bi = b0 + j
                vt = vts[j]
                for qc, (qo, qw) in enumerate(CHg):
                    tps = ps1.tile([P, 512], F32, tag="ps1")
                    tview = tps[:, :396].rearrange("p (h c) -> p h c", c=66)
                    for hh in range(6):
                        for c, (off, wd) in enumerate(CHg):
                            nc.tensor.matmul(
                                tview[:qw, hh, :],
                                ptss[j][c][:wd, hh,
                                           QSHg[j] + qo:QSHg[j] + qo + qw],
                                vt[:wd, c, hh * 66:hh * 66 + 66],
                                start=(c == 0), stop=(c == nch - 1))
                    r = spool.tile([P, 6], F32, tag="rr")
                    nc.vector.reciprocal(r[:qw, :], tview[:qw, :, 64])
                    ot = opool.tile([P, 6, 64], F32R, tag="ot")
                    rap = r[:qw, :]
                    rb = bass.AP(rap.tensor, rap.offset,
                                 [list(x) for x in rap.ap] + [[0, 64]])
                    nc.vector.tensor_tensor(ot[:qw, :, :], tview[:qw, :, :64],
                                            rb, OP.mult)
                    # prune scores: weighted CLS column of exp tiles
                    if li in PRUNE and qc == 0:
                        wb = prpool.tile([P, 6], F32, tag="wb")
                        nc.gpsimd.partition_broadcast(wb[:], r[0:1, :])
                        sc = prpool.tile([P, 2], F32, tag="sc")
                        for c, (off, wd) in enumerate(CHg):
                            t6 = prpool.tile([P, 6], F32, tag="t6")
                            nc.vector.tensor_tensor(
                                t6[:wd, :], e0s[j][c][:wd, :], wb[:wd, :],
                                OP.mult)
                            nc.vector.reduce_sum(
                                sc[:wd, c:c + 1], t6[:wd, :],
                                axis=mybir.AxisListType.X)
                            nc.sync.dma_start(
                                sc_dram.ap()[bi, c * 128:c * 128 + wd],
                                sc[:wd, c])
                    # O^T -> feature-major via PE, then proj chunk
                    ofm = opool.tile([P, 3, P], F32R, tag="ofm")
                    pt2 = pstr.tile([P, 512], F32, tag="pstr")
                    pt2r = pt2.bitcast(F32R)
                    for f in range(3):
                        nc.tensor.transpose(
                            pt2r[:, f * P:f * P + qw],
                            ot[:qw, :, :].rearrange("p h d -> p (h d)")
                            [:, f * P:(f + 1) * P],
                            ident[:qw, :qw])
                    srco = pt2r[:, :384].rearrange("p (f w) -> p f w", w=P)[:, :, :qw]
                    if (j + qc) % 2 == 0:
                        nc.vector.tensor_copy(ofm[:, :, :qw], srco)
                    else:
                        nc.scalar.copy(ofm[:, :, :qw], srco)
                    pj = ps1.tile([P, 512], F32, tag="ps1")
                    acc = pj[:qw, :C]
                    for kt in range(3):
                        nc.tensor.matmul(acc, ofm[:, kt, :qw], wp_t[:, kt, :],
                                         start=(kt == 0), stop=(kt == 2))
                    yb = apool.tile([P, C], F32, tag="yb")
                    nc.vector.tensor_tensor(yb[:qw, :], acc, bp_t[:qw, :],
                                            OP.add)
                    nc.vector.tensor_tensor(h[:qw, bi, qc, :],
                                            h[:qw, bi, qc, :], yb[:qw, :],
                                            OP.add)

        def mlp_pair(p, CHg, Wg, xfm2, w1_t, b1f_t, w2_t, b2_t):
            b0 = 2 * p
            PWg = 2 * Wg
            g = gpool.tile([P, 12, PW_MAX], F16, tag="g")
            for m in range(12):
                f1 = ps1.tile([P, 512], F32, tag="ps1")
                acc = f1[:, :PWg]
                for kt in range(3):
                    nc.tensor.matmul(
                        acc, w1_t[:, kt, m * P:(m + 1) * P],
                        xfm2[:, kt, :PWg],
                        start=(kt == 0), stop=(kt == 2))
                nc.scalar.activation(g[:, m, :PWg], acc, AF.Gelu,
                                     bias=b1f_t[:, m:m + 1])
            for j in (0, 1):
                bi = b0 + j
                for c, (off, wd) in enumerate(CHg):
                    span = j * Wg + off
                    f2 = ps1.tile([P, 512], F32, tag="ps1")
                    acc = f2[:wd, :C]
                    for kt in range(12):
                        nc.tensor.matmul(acc, g[:, kt, span:span + wd],
                                         w2_t[:, kt, :],
                                         start=(kt == 0), stop=(kt == 11))
                    yb = apool.tile([P, C], F32, tag="yb")
                    nc.vector.tensor_tensor(yb[:wd, :], acc, b2_t[:wd, :],
                                            OP.add)
                    nc.vector.tensor_tensor(h[:wd, bi, c, :],
                                            h[:wd, bi, c, :], yb[:wd, :],
                                            OP.add)

        def mask_update(li):
            drop = PRUNE[li]
            scm = prpool.tile([IMGS, 256], F32, tag="scm")
            nc.sync.dma_start(scm[:], sc_dram.ap())
            # t = -1e9*(sc==0) - sc  over tokens 1..196
            tneg = prpool.tile([IMGS, 256], F32, tag="tneg")
            u = prpool.tile([IMGS, 256], F32, tag="uu")
            nc.vector.tensor_scalar(u[:, 1:NTOK], scm[:, 1:NTOK], 0.0, None,
                                    OP.is_equal)
            nc.vector.scalar_tensor_tensor(
                tneg[:, 1:NTOK], u[:, 1:NTOK], -1e9, scm[:, 1:NTOK],
                OP.mult, OP.subtract)
            m8 = prpool.tile([IMGS, 8], F32, tag="m8")
            left = drop
            while left > 0:
                k = min(8, left)
                nc.vector.max(m8[:], tneg[:, 1:NTOK])
                if k < 8:
                    nc.vector.memset(m8[:, k:], -2e30)
                nc.vector.match_replace(tneg[:, 1:NTOK], m8[:],
                                        tneg[:, 1:NTOK], NEG)
                left -= k
            dm = prpool.tile([IMGS, 256], F32, tag="dm")
            nc.vector.memset(dm[:], 0.0)
            nc.vector.tensor_scalar(dm[:, 1:NTOK], tneg[:, 1:NTOK], -1e29,
                                    None, OP.is_le)
            nc.sync.dma_start(dm_dram.ap(), dm[:])
            dmc = prpool.tile([P, IMGS, 2], F32, tag="dmc")
            for b in range(IMGS):
                nc.sync.dma_start(
                    dmc[:, b, :],
                    bass.AP(dm_dram, b * 256, [[1, 128], [128, 2]]))
            nc.vector.scalar_tensor_tensor(mb[:], dmc[:], NEG, mb[:],
                                           OP.mult, OP.add)

        def compact():
            """Gather the 120 kept tokens into chunk 0 (partitions 0..119)."""
            kq = prpool.tile([P, IMGS, 2], F32, tag="kq")
            nc.vector.tensor_scalar(kq[:], mb[:], 0.0, None, OP.is_equal)
            kqr = kq
            # ranks (inclusive prefix sums over token index) via PE
            rr = prpool.tile([P, IMGS, 2], F32, tag="rrk")
            pr0 = ps1.tile([P, 512], F32, tag="ps1")
            nc.tensor.matmul(pr0[:128, :IMGS], trU[:, :], kqr[:, :, 0],
                             start=True, stop=True)
            nc.vector.tensor_copy(rr[:, :, 0], pr0[:128, :IMGS])
            pr1 = ps1.tile([P, 512], F32, tag="ps1")
            nc.tensor.matmul(pr1[:80, :IMGS], ones128[:, :80], kqr[:, :, 0],
                             start=True, stop=False)
            nc.tensor.matmul(pr1[:80, :IMGS], trU[:80, :80], kqr[:80, :, 1],
                             start=False, stop=True)
            nc.vector.tensor_copy(rr[:80, :, 1], pr1[:80, :IMGS])
            for b in range(IMGS):
                ptb = prpool.tile([P, 2, P], F32, tag="ptb")
                for c in range(2):
                    # PT[s, j] = (j+1 == rank_s) * keep_s
                    nc.vector.tensor_scalar(
                        ptb[:, c, :], io1b[:, :], rr[:, b, c:c + 1],
                        kq[:, b, c:c + 1], OP.is_equal, OP.mult)
                pg = ps1.tile([P, 512], F32, tag="ps1")
                acc = pg[:, :C]
                nc.tensor.matmul(acc, ptb[:, 0, :], h[:, b, 0, :],
                                 start=True, stop=False)
                nc.tensor.matmul(acc, ptb[:80, 1, :], h[:80, b, 1, :],
                                 start=False, stop=True)
                nc.vector.tensor_copy(h[:, b, 0, :], acc)
            # rebuild key bias for the compacted layout (static):
            # 0 for partitions < NKEEP, NEG for the zero-padded tail
            nc.vector.memset(mb[:, :, 0], 0.0)
            nc.gpsimd.affine_select(
                out=mb[:, :, 0], in_=mb[:, :, 0], pattern=[[0, IMGS]],
                compare_op=OP.is_ge, fill=NEG, base=NKEEP - 1,
                channel_multiplier=-1)

        # ---------------- transformer layers ----------------
        # Software pipeline: LN (DVE stats + ACT normalize + PE transpose)
        # for compute-block k+2 is emitted before compute-block k, so the
        # LN chain of the next pair runs on DVE/ACT while the PE chews on
        # the current pair's matmuls.
        PW_MAX = 2 * W_A

        def geom_of(li):
            if li <= COMPACT_AT or NO_COMPACT:
                return CH_A, W_A, QWIN_A, QSH_A
            return CH_B, W_B, QWIN_B, QSH_B

        blocks = []
        for li in range(n_layers):
            for p in range(PAIRS):
                blocks.append(("A", li, p))
            for p in range(PAIRS):
                blocks.append(("M", li, p))

        xfms = {}
        stats_d = {}
        pend = {}

        def emit_stats_for(blk):
            kind, li, p = blk
            CHg, _, _, _ = geom_of(li)
            stats_d[blk] = ln_stats(p, CHg)

        def emit_ln_for(blk):
            kind, li, p = blk
            CHg, Wg, _, _ = geom_of(li)
            if blk not in stats_d:
                emit_stats_for(blk)
            rstd, nmean = stats_d.pop(blk)
            x = xpool.tile([P, 3, PW_MAX], F32R, tag="xfm")
            ln_apply(p, CHg, Wg, x, rstd, nmean, to_dve=(kind == "M"))
            xfms[blk] = x

        def ln_blocked(cur, tgt):
            # LN of post-compaction blocks can't be emitted pre-compaction
            return (not NO_COMPACT) and tgt[1] > COMPACT_AT and cur[1] <= COMPACT_AT

        weights = {}

        def load_weights(li):
            wqk_t = wpool.tile([P, 3, 768], F32R, tag="wqk")
            nc.sync.dma_start(wqk_t[:], wqk.ap()[li].rearrange("k p m -> p k m"))
            bqk_t = wpool.tile([P, 6], F32, tag="bqk")
            nc.sync.dma_start(bqk_t[:], bqk.ap()[li].rearrange("m p -> p m"))
            wv_t = wpool1.tile([P, 3, 396], F32R, tag="wv")
            nc.sync.dma_start(wv_t[:], wv.ap()[li].rearrange("k p m -> p k m"))
            bv_t = wpool1.tile([1, 396], F32, tag="bv")
            nc.sync.dma_start(bv_t[:], bv.ap()[li])
            bv_m = wpool1.tile([P, 396], F32, tag="bvm")
            nc.gpsimd.partition_broadcast(bv_m[:], bv_t[:])
            wp_t = wpool1.tile([P, 3, C], F32R, tag="wp")
            nc.sync.dma_start(wp_t[:], wp.ap()[li].rearrange("k p m -> p k m"))
            bp_t = wpool1.tile([1, C], F32R, tag="bp")
            nc.sync.dma_start(bp_t[:], bp.ap()[li])
            bp_m = wpool1.tile([P, C], F32, tag="bpm")
            nc.gpsimd.partition_broadcast(bp_m[:], bp_t[:].bitcast(F32))
            w1_t = wpool1.tile([P, 3, MLP], F32R, tag="w1")
            for q4 in range(4):
                nc.sync.dma_start(
                    w1_t[:, :, q4 * 384:(q4 + 1) * 384],
                    w1.ap()[li].rearrange("k p m -> p k m")[:, :, q4 * 384:(q4 + 1) * 384])
            b1f_t = wpool1.tile([P, 12], F32, tag="b1")
            nc.sync.dma_start(b1f_t[:], b1.ap()[li].rearrange("o (m p) -> p (o m)", p=P))
            w2_t = wpool1.tile([P, 12, C], F16, tag="w2")
            for q4 in range(4):
                nc.sync.dma_start(
                    w2_t[:, q4 * 3:(q4 + 1) * 3, :],
                    w2.ap()[li].rearrange("k p m -> p k m")[:, q4 * 3:(q4 + 1) * 3, :])
            b2_t = wpool1.tile([1, C], F16, tag="b2")
            nc.sync.dma_start(b2_t[:], b2.ap()[li])
            b2_m = wpool1.tile([P, C], F16, tag="b2m")
            nc.gpsimd.partition_broadcast(b2_m[:], b2_t[:])
            weights[li] = (wqk_t, bqk_t, wv_t, bv_m, wp_t, bp_m,
                           w1_t, b1f_t, w2_t, b2_m)

        emit_stats_for(blocks[0])
        if len(blocks) > 1:
            emit_stats_for(blocks[1])
        emit_ln_for(blocks[0])
        if len(blocks) > 2:
            emit_stats_for(blocks[2])
        if len(blocks) > 1:
            emit_ln_for(blocks[1])
        for k, blk in enumerate(blocks):
            kind, li, p = blk
            CHg, Wg, QWINg, QSHg = geom_of(li)
            if blk not in xfms:          # deferred (post-compaction) LN
                emit_ln_for(blk)
            if kind == "A" and p == 0:
                load_weights(li)
            (wqk_t, bqk_t, wv_t, bv_m, wp_t, bp_t,
             w1_t, b1f_t, w2_t, b2_t) = weights[li]
            xfm = xfms.pop(blk)
            if kind == "A":
                pend[p] = attn_h1(li, p, CHg, Wg, QWINg, QSHg, xfm, wqk_t,
                                  bqk_t, wv_t, bv_m)
                if p > 0:
                    attn_h2(li, p - 1, CHg, Wg, QWINg, QSHg, pend.pop(p - 1),
                            wp_t, bp_t)
                if p == PAIRS - 1:
                    attn_h2(li, p, CHg, Wg, QWINg, QSHg, pend.pop(p),
                            wp_t, bp_t)
            else:
                mlp_pair(p, CHg, Wg, xfm, w1_t, b1f_t, w2_t, b2_t)
                if p == 0 and li in PRUNE:
                    mask_update(li)
                if (p == PAIRS - 1 and li == COMPACT_AT and not NO_COMPACT
                        and n_layers > COMPACT_AT + 1):
                    compact()
            if k + 3 < len(blocks) and not ln_blocked(blk, blocks[k + 3]):
                emit_stats_for(blocks[k + 3])
            if k + 2 < len(blocks) and not ln_blocked(blk, blocks[k + 2]):
                emit_ln_for(blocks[k + 2])

        # ---------------- final LN + head ----------------
        for b in range(IMGS):
            nc.sync.dma_start(cls_dram.ap()[b, :], h[0:1, b, 0, :])
        clst = prpool.tile([IMGS, C], F32, tag="clst")
        nc.sync.dma_start(clst[:], cls_dram.ap())
        s6 = prpool.tile([IMGS, 6], F32, tag="s6f")
        mv = prpool.tile([IMGS, 2], F32, tag="mvf")
        nc.vector.bn_stats(s6[:], clst[:])
        nc.vector.bn_aggr(mv[:], s6[:])
        rstd = prpool.tile([IMGS, 1], F32, tag="rstdf")
        _rsqrt(nc, spool, rstd, mv[:, 1:2], EPS)
        nmean = prpool.tile([IMGS, 1], F32, tag="nmeanf")
        nc.vector.scalar_tensor_tensor(nmean[:], mv[:, 0:1], -1.0, rstd[:],
                                       OP.mult, OP.mult)
        clsn = prpool.tile([IMGS, C], F32R, tag="clsn")
        nc.scalar.activation(clsn[:], clst[:], AF.Identity, bias=nmean[:],
                             scale=rstd[:])
        clsf = prpool.tile([P, 3, IMGS], F32R, tag="clsf")
        for f in range(3):
            pt = pstr.tile([P, 512], F32, tag="pstr")
            ptr = pt.bitcast(F32R)
            nc.tensor.transpose(ptr[:, :IMGS], clsn[:, f * P:(f + 1) * P],
                                ident[:IMGS, :IMGS])
            nc.vector.tensor_copy(clsf[:, f, :], ptr[:, :IMGS])
        wh_t = prpool.tile([P, 3, NCLS], F32R, tag="wht")
        nc.sync.dma_start(wh_t[:], wh.ap().rearrange("k p m -> p k m"))
        bh_t = prpool.tile([1, NCLS], F32R, tag="bht")
        nc.sync.dma_start(bh_t[:], bh.ap())
        po = ps1.tile([P, 512], F32, tag="ps1")
        acc = po[:IMGS, :NCLS]
        for kt in range(3):
            nc.tensor.matmul(acc, clsf[:, kt, :], wh_t[:, kt, :],
                             start=(kt == 0), stop=False)
        nc.tensor.matmul(acc, ones_r[:, :IMGS], bh_t[:], start=False, stop=True)
        ot = prpool.tile([IMGS, NCLS], F32, tag="outf")
        nc.vector.tensor_copy(ot[:], acc)
        nc.sync.dma_start(out.ap(), ot[:])

    nc.finalize()
    return nc


# ======================= host side =======================

def _prep(inputs):
    """Host-side: patchify x, fold LN affines, lay out weights."""
    f32 = np.float32
    f16 = np.float16
    d = {}
    x = np.asarray(inputs["x"], f32)
    Bn = x.shape[0]
    # patches feature-major, with token shift (col 0 = CLS placeholder)
    p = x.reshape(Bn, 3, 14, 16, 14, 16).transpose(0, 2, 4, 1, 3, 5)
    p = p.reshape(Bn, NPATCH, 768)
    xp = np.zeros((Bn, 768, W_A), f32)
    xp[:, :, 1:NTOK] = p.transpose(0, 2, 1)
    d["xp_all"] = xp.reshape(Bn, 6, 128, W_A)

    pw_ = np.asarray(inputs["patch_w"], f32)
    d["pw"] = pw_.reshape(6, 128, C)

    h0b = np.zeros((2, 128, C), f32)
    pos = np.asarray(inputs["pos_embed"], f32)[0]
    pb = np.asarray(inputs["patch_b"], f32)
    cls0 = np.asarray(inputs["cls_token"], f32).reshape(C) + pos[0]
    bias_tok = np.zeros((W_A, C), f32)
    bias_tok[0] = cls0
    bias_tok[1:NTOK] = pb[None, :] + pos[1:]
    for c, (off, wd) in enumerate(CH_A):
        h0b[c, :wd] = bias_tok[off:off + wd]
    d["h0b"] = h0b

    mb_ = np.zeros((2, 128), f32)
    for c, (off, wd) in enumerate(CH_A):
        for pp in range(128):
            t = off + pp
            if pp >= wd or t >= NTOK:
                mb_[c, pp] = NEG
    d["mb0"] = mb_

    qkv_w = np.asarray(inputs["qkv_w"], f32)
    qkv_b = np.asarray(inputs["qkv_b"], f32)
    g1 = np.asarray(inputs["ln1_g"], f32)
    b1_ = np.asarray(inputs["ln1_b"], f32)
    g2 = np.asarray(inputs["ln2_g"], f32)
    b2_ = np.asarray(inputs["ln2_b"], f32)

    wqk_l = np.zeros((LAYERS, 3, 128, 768), f32)
    bqk_l = np.zeros((LAYERS, 6, 128), f32)
    wv_l = np.zeros((LAYERS, 3, 128, 396), f32)
    bv_l = np.zeros((LAYERS, 1, 396), f32)
    for li in range(LAYERS):
        wq = qkv_w[li] * g1[li][:, None]          # [C, 3C] folded
        bq = qkv_b[li] + b1_[li] @ qkv_w[li]
        wqk2 = np.zeros((C, 768), f32)
        for m in range(6):
            wqk2[:, m * 128:m * 128 + 64] = wq[:, m * 64:(m + 1) * 64]
            wqk2[:, m * 128 + 64:m * 128 + 128] = \
                wq[:, 384 + m * 64:384 + (m + 1) * 64]
            bqk_l[li, m, 0:64] = bq[m * 64:(m + 1) * 64]
            bqk_l[li, m, 64:128] = bq[384 + m * 64:384 + (m + 1) * 64]
        wqk_l[li] = wqk2.reshape(3, 128, 768)
        wvl = np.zeros((C, 396), f32)
        bvl = np.zeros((396,), f32)
        for hh in range(HEADS):
            wvl[:, hh * 66:hh * 66 + 64] = wq[:, 768 + hh * 64:768 + (hh + 1) * 64]
            bvl[hh * 66:hh * 66 + 64] = bq[768 + hh * 64:768 + (hh + 1) * 64]
            bvl[hh * 66 + 64] = 1.0
        wv_l[li] = wvl.reshape(3, 128, 396)
        bv_l[li, 0] = bvl
    d["wqk"], d["bqk"], d["wv"], d["bv"] = wqk_l, bqk_l, wv_l, bv_l

    d["wp"] = np.asarray(inputs["proj_w"], f32).reshape(LAYERS, 3, 128, C)
    d["bp"] = np.asarray(inputs["proj_b"], f32).reshape(LAYERS, 1, C)
    w1_ = np.asarray(inputs["fc1_w"], f32) * g2[:, :, None]
    d["w1"] = w1_.reshape(LAYERS, 3, 128, MLP)
    d["b1"] = (np.asarray(inputs["fc1_b"], f32)
               + np.einsum('lc,lcm->lm', b2_, np.asarray(inputs["fc1_w"], f32))
               ).reshape(LAYERS, 1, MLP)
    d["w2"] = np.asarray(inputs["fc2_w"], f32).reshape(
        LAYERS, 12, 128, C).astype(f16)
    d["b2"] = np.asarray(inputs["fc2_b"], f32).reshape(
        LAYERS, 1, C).astype(f16)

    ng = np.asarray(inputs["norm_g"], f32)
    nb = np.asarray(inputs["norm_b"], f32)
    hw = np.asarray(inputs["head_w"], f32)
    d["wh"] = (hw * ng[:, None]).reshape(3, 128, NCLS)
    d["bh"] = (np.asarray(inputs["head_b"], f32) + nb @ hw).reshape(1, NCLS)
    return d


_NC_CACHE = {}


def kernel(**inputs):
    key = (N_LAYERS_BUILD, NO_COMPACT)
    if key not in _NC_CACHE:
        _NC_CACHE[key] = build_kernel()
    nc = _NC_CACHE[key]
    d = _prep(inputs)
    shared = {k: np.ascontiguousarray(v) for k, v in d.items() if k != "xp_all"}
    in_maps = []
    for core in range(8):
        m = dict(shared)
        m["xp"] = np.ascontiguousarray(
            d["xp_all"][core * IMGS:(core + 1) * IMGS].transpose(1, 2, 0, 3))
        in_maps.append(m)
    res = run_bass_kernel_spmd(nc, in_maps, core_ids=list(range(8)))
    outs = [r["out"] for r in res.results]
    return np.concatenate(outs, axis=0)


if __name__ == "__main__":
    print("building kernel ...")
    nc = build_kernel()
    print("built OK")
